# revision 25
# baseline (speedup 1.0000x reference)
"""HTSK fuzzy-system kernel for Trainium2 (Bass/Tile), 8-core data-parallel.

Math (per batch row b):
  S     = H/sigma^2 + EPS                          (D,R)
  m     = mean_d(-(X_bd - C_dr)^2 * S_dr)          (B,R)
        = X^2 @ (-S/D) + X @ (2*S*C/D) + K2        (matmul expansion)
  e     = exp(m)            (unnormalized softmax; m is bounded, no max needed)
  out   = (1/sum_r e) * ( sum_r e_br * G_bro  +  e @ (W2 + 1 b^T) )
  G     = X @ Wt,  Wt[d, h*4096 + o*64 + rr] = W[(h*64+rr)*D+d, o]

G layout: r split in low/high halves (h), o-major within each, rr innermost.
- innermost rr is step-1 so the e-broadcast multiply runs in DVE 2x_1P mode
- the r-halves live in two separate contiguous 4KB-per-partition tiles, so
  tree level 1 is ONE contiguous SBUF->SBUF DMA with the CCE inline adder
  (accum_op=add), running on the otherwise-idle DMA engines
When sigmas are uniform the X^2 term is constant over r and cancels in the
softmax, so the A-matmuls are dropped. sum_r e comes from a ones column
appended to W2.

Two phases per core:
  1) logits/exp/e-transpose/out2 for all 4 row-tiles, using 3 PSUM banks
     (scoped pools), overlapped with the Wt DMA stream
  2) G phase: all 8 PSUM banks as 2x[128,2048] fp32 ping-pong; per
     pair-chunk 8 matmuls (stationary changes once), ScalarE drain to bf16,
     DVE 2x multiply; tree L1 on DMA (CCE), L2..L7 + epilogue on DVE

Sharding: batch B=4096 split 512 rows per core; weights replicated.
All small constants + X^T ship in one packed [128, x] DMA blob per core.
"""
import os
import sys
import types

import numpy as np

sys.path.insert(0, "/opt/trn_rl_repo")

# NTFF profile-hook registry: trn_boot sets it at jax init, concourse
# bass_utils reads it when trace=True. The container's antenv package lacks
# this submodule, so provide it before anything imports jax/concourse.
if "antenv.axon_hooks" not in sys.modules:
    _ah = types.ModuleType("antenv.axon_hooks")
    _ah._hook = None

    def _set_hook(hook):
        _ah._hook = hook

    def _get_hook():
        return _ah._hook

    _ah.set_axon_ntff_profile_hook = _set_hook
    _ah.get_axon_ntff_profile_hook = _get_hook
    sys.modules["antenv.axon_hooks"] = _ah

import ml_dtypes  # noqa: E402
import concourse.bass as bass  # noqa: E402
import concourse.bacc as bacc  # noqa: E402
import concourse.tile as tile  # noqa: E402
from contextlib import ExitStack  # noqa: E402
from concourse import mybir  # noqa: E402
from concourse import bass_utils  # noqa: E402
from concourse.masks import make_identity  # noqa: E402

H = 0.5
EPS = 1e-8
B, D, R, O = 4096, 256, 128, 64
NCORES = 8
BL = B // NCORES          # 512 batch rows per core
NT = BL // 128            # 4 partition tiles per core
RO = R * O                # 8192 G columns per row
F32 = mybir.dt.float32
BF16 = mybir.dt.bfloat16
EXPF = mybir.ActivationFunctionType.Exp
ADD = mybir.AluOpType.add

# const blob column offsets (bf16 elements)
OBM, OK2, OW2, OXT = 0, 256, 384, 520
NC_BASE = OXT + 1024          # 1544
OA = NC_BASE                  # A appended when sigmas are non-uniform

_CACHE = {}
LAST_RESULT = None
TRACE = False
TRACE_DIR = "/root/problem/work/trace_out"
# Level-1 tree via SDMA accum_op=add was tried and wedges the device: the
# SWDGE descriptor generator (GpSimd) deadlocks against DVE's shared SBUF
# port (exclusive lock). Keep the level-1 add on DVE.
L1_DMA = os.environ.get("L1_DMA", "0") == "1"


def _build(use_a: bool):
    ncols = NC_BASE + (256 if use_a else 0)
    nc = bacc.Bacc("TRN2", target_bir_lowering=False, debug=False)
    CONST = nc.dram_tensor("CONST", [128, ncols], BF16, kind="ExternalInput")
    WT = nc.dram_tensor("WT", [D, RO], BF16, kind="ExternalInput")
    out = nc.dram_tensor("out", [BL, O], F32, kind="ExternalOutput")

    with tile.TileContext(nc) as tc, ExitStack() as ctx:
        consts = ctx.enter_context(tc.tile_pool(name="consts", bufs=1))
        work = ctx.enter_context(tc.tile_pool(name="work", bufs=2))
        gsbp = ctx.enter_context(tc.tile_pool(name="gsb", bufs=2))
        gap = ctx.enter_context(tc.tile_pool(name="ga", bufs=2))
        gbp = ctx.enter_context(tc.tile_pool(name="gb", bufs=2))
        treep = ctx.enter_context(tc.tile_pool(name="tree", bufs=2))

        # ---- packed const DMA split across both HWDGE queues; Wt streams
        # on sync+gpsimd behind it ----
        conc = consts.tile([128, ncols], BF16, tag="conc")
        half = (ncols // 2) & ~3
        nc.sync.dma_start(out=conc[:, 0:half], in_=CONST[:, 0:half])
        nc.scalar.dma_start(out=conc[:, half:], in_=CONST[:, half:])
        bm_sb = conc[:, OBM:OBM + 256].rearrange("p (c r) -> p c r", r=R)
        k2_sb = conc[0:1, OK2:OK2 + R]
        w2b_sb = conc[:, OW2:OW2 + O + 1]          # [R, 65]: W2+b ++ ones col
        xTv = conc[:, OXT:OXT + 1024].rearrange("p (c j) -> p c j", j=BL)
        if use_a:
            a_sb = conc[:, OA:OA + 256].rearrange("p (c r) -> p c r", r=R)
        identB = consts.tile([128, 128], BF16, tag="idb")
        make_identity(nc, identB)
        ones_sb = consts.tile([1, 128], BF16, tag="ones")
        nc.vector.memset(ones_sb, 1.0)
        # Wt pair-chunk tiles: c0 on sync ring, c1 on gpsimd (SWDGE) ring
        wt_sb = [[None] * 4, [None] * 4]
        for q in range(4):
            for c in range(2):
                t_ = consts.tile([128, 2048], BF16, tag=f"wt{c}{q}")
                eng = nc.sync if c == 0 else nc.gpsimd
                eng.dma_start(out=t_[:, :],
                              in_=WT[c * 128:(c + 1) * 128,
                                     q * 2048:(q + 1) * 2048])
                wt_sb[c][q] = t_
        if use_a:
            x2T = consts.tile([128, 2, BL], BF16, tag="x2T")
            for c in range(2):
                nc.scalar.square(x2T[:, c, :], xTv[:, c, :])

        # per-tile softmax state, alive through phase 2
        e_bf = [consts.tile([128, R], BF16, tag=f"e{t}", name=f"e_{t}")
                for t in range(NT)]
        rs_all = consts.tile([128, NT], F32, tag="rs")
        o2_sb = consts.tile([128, NT * (O + 1)], F32, tag="o2sb")

        # ---- phase 1: logits / exp / e^T / out2 for all tiles ----
        with tc.tile_pool(name="ps_pre", bufs=1, space="PSUM") as ps_pre, \
             tc.tile_pool(name="ps_eT", bufs=2, space="PSUM") as ps_eT:
            m_ps = [ps_pre.tile([128, R], F32, tag=f"m{t}", name=f"m_{t}")
                    for t in range(NT)]                           # 1 bank each
            for t in range(NT):
                bs = slice(t * 128, (t + 1) * 128)
                mt = m_ps[t]
                first = True
                if use_a:
                    for c in range(2):
                        nc.tensor.matmul(mt, lhsT=x2T[:, c, bs],
                                         rhs=a_sb[:, c, :],
                                         start=first, stop=False)
                        first = False
                for c in range(2):
                    nc.tensor.matmul(mt, lhsT=xTv[:, c, bs], rhs=bm_sb[:, c, :],
                                     start=first, stop=False)
                    first = False
                nc.tensor.matmul(mt, lhsT=ones_sb, rhs=k2_sb,
                                 start=False, stop=True)
            for t in range(NT):
                nc.scalar.activation(e_bf[t], m_ps[t], EXPF, bias=0.0, scale=1.0)
                eT_ps = ps_eT.tile([128, 128], BF16, tag="eT")
                nc.tensor.transpose(eT_ps, e_bf[t], identB)
                eT_sb = work.tile([128, 128], BF16, tag="eTsb")
                nc.vector.tensor_copy(eT_sb, eT_ps)
                # out2 reuses tile t's freed m bank (m was consumed by exp)
                nc.tensor.matmul(m_ps[t][:, 0:O + 1], lhsT=eT_sb,
                                 rhs=w2b_sb, start=True, stop=True)
                nc.vector.tensor_copy(o2_sb[:, t * 65:(t + 1) * 65],
                                      m_ps[t][:, 0:O + 1])
                nc.vector.reciprocal(rs_all[:, t:t + 1],
                                     o2_sb[:, t * 65 + O:t * 65 + O + 1])
                # pre-normalize: e <- e/sum_e and out2 <- out2/sum_e, so the
                # per-tile epilogue is a plain add (no tensor_scalar_mul)
                nc.scalar.mul(e_bf[t], e_bf[t], rs_all[:, t:t + 1])
                nc.vector.tensor_scalar_mul(
                    o2_sb[:, t * 65:t * 65 + O], o2_sb[:, t * 65:t * 65 + O],
                    rs_all[:, t:t + 1])
                # filler matmuls pad the exp-gated PE gaps so HAM stays at
                # full clock when the G matmuls begin (cold-isolated MMs cost
                # 634ns vs 215ns warm)
                warm = ps_eT.tile([128, R], F32, tag="warm", name=f"warm_{t}")
                for w in range(3):
                    nc.tensor.matmul(warm, lhsT=xTv[:, w % 2, 0:128],
                                     rhs=bm_sb[:, w % 2, :],
                                     start=True, stop=True)

        # ---- phase 2: G matmuls, drains, multiplies, tree ----
        # Emission is software-pipelined: tile t's tree/epilogue instructions
        # are emitted AFTER tile t+1's drains/multiplies so the DVE FIFO never
        # blocks the next tile's multiplies behind a finished tile's tree.
        def reduce_half(z3, t, obase, n, sub):
            # z3: [128, n, 64] view whose o-axis starts at absolute o=obase
            t2 = treep.tile([128, n, 32], BF16, tag=f"t2{sub}",
                            name=f"t2_{t}{sub}")
            nc.vector.tensor_add(t2, z3[:, :, 0:32], z3[:, :, 32:64])
            t3 = treep.tile([128, n, 16], BF16, tag=f"t3{sub}",
                            name=f"t3_{t}{sub}")
            nc.vector.tensor_add(t3, t2[:, :, 0:16], t2[:, :, 16:32])
            t4 = treep.tile([128, n, 8], BF16, tag=f"t4{sub}",
                            name=f"t4_{t}{sub}")
            nc.vector.tensor_add(t4, t3[:, :, 0:8], t3[:, :, 8:16])
            t5 = treep.tile([128, n, 4], BF16, tag=f"t5{sub}",
                            name=f"t5_{t}{sub}")
            nc.vector.tensor_add(t5, t4[:, :, 0:4], t4[:, :, 4:8])
            t6 = treep.tile([128, n, 2], BF16, tag=f"t6{sub}",
                            name=f"t6_{t}{sub}")
            nc.vector.tensor_add(t6, t5[:, :, 0:2], t5[:, :, 2:4])
            red = work.tile([128, n, 1], BF16, tag=f"red{sub}",
                            name=f"red_{t}{sub}")
            nc.vector.tensor_add(red, t6[:, :, 0:1], t6[:, :, 1:2])
            osb = work.tile([128, n], F32, tag=f"osb{sub}", name=f"osb_{t}{sub}")
            nc.vector.tensor_add(osb, red.rearrange("p o () -> p o"),
                                 o2_sb[:, t * 65 + obase:t * 65 + obase + n])
            nc.sync.dma_start(out=out[t * 128:(t + 1) * 128, obase:obase + n],
                              in_=osb)

        with tc.tile_pool(name="ps_g", bufs=2, space="PSUM") as ps_g:
            state = {}

            def emit_G(t):
                bs = slice(t * 128, (t + 1) * 128)
                last = t == NT - 1
                gsb = gsbp.tile([128, RO], BF16, tag="gsb", name=f"gsb_{t}")
                ga = gap.tile([128, 4096], BF16, tag="ga", name=f"ga_{t}")
                gb = gbp.tile([128, 4096], BF16, tag="gb", name=f"gb_{t}")
                ga3 = ga.rearrange("p (o r) -> p o r", r=64)
                gb3 = gb.rearrange("p (o r) -> p o r", r=64)
                gsb3 = gsb.rearrange("p (o r) -> p o r", r=64)
                # last tile: o-low pair-chunks first so its first half-tree
                # can start while the o-high matmuls still run
                order = (0, 2, 1, 3) if last else (0, 1, 2, 3)
                for i, pq in enumerate(order):
                    gt = ps_g.tile([128, 2048], F32, tag="g", name=f"g_{t}_{pq}")
                    for c in range(2):
                        for h in range(4):
                            nc.tensor.matmul(
                                gt[:, h * 512:(h + 1) * 512],
                                lhsT=xTv[:, c, bs],
                                rhs=wt_sb[c][pq][:, h * 512:(h + 1) * 512],
                                start=(c == 0), stop=(c == 1),
                            )
                    half, oq = divmod(pq, 2)
                    if last and i == 3:
                        # final chunk of the run: multiply straight from PSUM
                        # (1x) to skip the ScalarE drain on the critical tail
                        dst3 = (ga3 if half == 0
                                else gb3)[:, oq * 32:(oq + 1) * 32, :]
                        ebc = (e_bf[t][:, half * 64:(half + 1) * 64]
                               .rearrange("p r -> p () r")
                               .broadcast_to((128, 32, 64)))
                        nc.vector.tensor_mul(
                            dst3,
                            gt.rearrange("p (o r) -> p o r", r=64), ebc)
                        continue
                    nc.scalar.copy(gsb[:, pq * 2048:(pq + 1) * 2048], gt)
                    ebch = (e_bf[t][:, half * 64:(half + 1) * 64]
                            .rearrange("p r -> p () r"))
                    if last:
                        # keep per-chunk multiplies: the o-low half-tree
                        # starts as soon as pair-chunks 0 and 2 are done
                        dst3 = (ga3 if half == 0
                                else gb3)[:, oq * 32:(oq + 1) * 32, :]
                        nc.vector.tensor_mul(
                            dst3, gsb3[:, pq * 32:(pq + 1) * 32, :],
                            ebch.broadcast_to((128, 32, 64)))
                    elif pq % 2 == 1:
                        # both o-halves of this r-half drained: one merged
                        # 4096-wide 2x multiply (fewer DVE op overheads)
                        dst3 = ga3 if half == 0 else gb3
                        nc.vector.tensor_mul(
                            dst3, gsb3[:, half * 64:half * 64 + 64, :],
                            ebch.broadcast_to((128, 64, 64)))
                    if last and i == 1:
                        # o 0:32 complete in both halves: first half-tree now
                        t1a = treep.tile([128, 32, 64], BF16, tag="t1a")
                        nc.vector.tensor_add(t1a, ga3[:, 0:32, :],
                                             gb3[:, 0:32, :])
                        reduce_half(t1a, t, 0, 32, "a")
                state["ga"], state["gb"] = ga3, gb3

            def emit_tree(t):
                ga3, gb3 = state["t%d" % t]
                if L1_DMA:
                    nc.gpsimd.dma_start(
                        out=ga3.rearrange("p o r -> p (o r)"),
                        in_=gb3.rearrange("p o r -> p (o r)"), accum_op=ADD)
                    z = ga3
                else:
                    t1 = treep.tile([128, 64, 64], BF16, tag="t1",
                                    name=f"t1_{t}")
                    nc.vector.tensor_add(t1, ga3, gb3)
                    z = t1
                reduce_half(z, t, 0, 64, "")  # full width: obase=0, n=64

            for t in range(NT):
                emit_G(t)
                state["t%d" % t] = (state["ga"], state["gb"])
                if t > 0:
                    emit_tree(t - 1)
            # last tile: second half-tree only (first half emitted inline)
            ga3, gb3 = state["t%d" % (NT - 1)]
            t1b = treep.tile([128, 32, 64], BF16, tag="t1b")
            nc.vector.tensor_add(t1b, ga3[:, 32:64, :], gb3[:, 32:64, :])
            reduce_half(t1b, NT - 1, 32, 32, "b")

    nc.finalize()
    return nc


def _get_nc(use_a: bool):
    key = ("nc", use_a)
    if key not in _CACHE:
        _CACHE[key] = _build(use_a)
    return _CACHE[key]


def _host_prep(centers, sigmas, W, b):
    c64 = centers.astype(np.float64)
    S = (H / sigmas.astype(np.float64) ** 2) + EPS          # (D,R)
    use_a = not np.allclose(S, S.flat[0])
    bf = ml_dtypes.bfloat16
    ncols = NC_BASE + (256 if use_a else 0)
    CB = np.zeros((128, ncols), dtype=bf)
    Bm = (2.0 * S * c64 / D).astype(bf)                      # X coeff
    CB[:, OBM:OBM + 128] = Bm[0:128]
    CB[:, OBM + 128:OBM + 256] = Bm[128:256]
    K2 = (-(S * c64 * c64).sum(axis=0) / D).astype(bf)
    CB[0, OK2:OK2 + R] = K2
    W2b = np.concatenate(
        [W[D * R:].astype(np.float64) + b[None, :].astype(np.float64),
         np.ones((R, 1))], axis=1
    ).astype(bf)
    CB[:, OW2:OW2 + O + 1] = W2b
    if use_a:
        A = (-S / D).astype(bf)
        CB[:, OA:OA + 128] = A[0:128]
        CB[:, OA + 128:OA + 256] = A[128:256]
    W1 = W[: D * R].reshape(2, 64, D, O)          # (half, rr, d, o)
    # split-half o-major: Wt[d, h*4096 + o*64 + rr] = W1[h, rr, d, o]
    Wt = np.ascontiguousarray(
        W1.transpose(2, 0, 3, 1).reshape(D, RO)).astype(bf)
    return use_a, CB, Wt


def kernel(X, centers, sigmas, W, b):
    global LAST_RESULT
    X = np.asarray(X, dtype=np.float32)
    centers = np.asarray(centers, dtype=np.float32)
    sigmas = np.asarray(sigmas, dtype=np.float32)
    W = np.asarray(W, dtype=np.float32)
    b = np.asarray(b, dtype=np.float32)

    use_a, CB, Wt = _host_prep(centers, sigmas, W, b)
    Xb = X.astype(ml_dtypes.bfloat16)
    nc = _get_nc(use_a)
    in_maps = []
    for k in range(NCORES):
        cb = CB.copy()
        xt = Xb[k * BL:(k + 1) * BL].T                      # (D, BL)
        cb[:, OXT:OXT + BL] = xt[0:128]
        cb[:, OXT + BL:OXT + 1024] = xt[128:256]
        in_maps.append({"CONST": cb, "WT": Wt})
    kw = {}
    if TRACE:
        import shutil
        shutil.rmtree(TRACE_DIR, ignore_errors=True)
        kw = {"trace": True, "tmpdir": TRACE_DIR}
    res = bass_utils.run_bass_kernel_spmd(
        nc, in_maps, core_ids=list(range(NCORES)), **kw
    )
    LAST_RESULT = res
    return np.concatenate([res.results[k]["out"] for k in range(NCORES)], axis=0)


# revision 27
# speedup vs baseline: 1.0122x; 1.0122x over previous
"""HTSK fuzzy-system kernel for Trainium2 (Bass/Tile), 8-core data-parallel.

Math (per batch row b):
  S     = H/sigma^2 + EPS                          (D,R)
  m     = mean_d(-(X_bd - C_dr)^2 * S_dr)          (B,R)
        = X^2 @ (-S/D) + X @ (2*S*C/D) + K2        (matmul expansion)
  e     = exp(m)            (unnormalized softmax; m is bounded, no max needed)
  out   = (1/sum_r e) * ( sum_r e_br * G_bro  +  e @ (W2 + 1 b^T) )
  G     = X @ Wt,  Wt[d, h*4096 + o*64 + rr] = W[(h*64+rr)*D+d, o]

G layout: r split in low/high halves (h), o-major within each, rr innermost.
- innermost rr is step-1 so the e-broadcast multiply runs in DVE 2x_1P mode
- the r-halves live in two separate contiguous 4KB-per-partition tiles, so
  tree level 1 is ONE contiguous SBUF->SBUF DMA with the CCE inline adder
  (accum_op=add), running on the otherwise-idle DMA engines
When sigmas are uniform the X^2 term is constant over r and cancels in the
softmax, so the A-matmuls are dropped. sum_r e comes from a ones column
appended to W2.

Two phases per core:
  1) logits/exp/e-transpose/out2 for all 4 row-tiles, using 3 PSUM banks
     (scoped pools), overlapped with the Wt DMA stream
  2) G phase: all 8 PSUM banks as 2x[128,2048] fp32 ping-pong; per
     pair-chunk 8 matmuls (stationary changes once), ScalarE drain to bf16,
     DVE 2x multiply; tree L1 on DMA (CCE), L2..L7 + epilogue on DVE

Sharding: batch B=4096 split 512 rows per core; weights replicated.
All small constants + X^T ship in one packed [128, x] DMA blob per core.
"""
import os
import sys
import types

import numpy as np

sys.path.insert(0, "/opt/trn_rl_repo")

# NTFF profile-hook registry: trn_boot sets it at jax init, concourse
# bass_utils reads it when trace=True. The container's antenv package lacks
# this submodule, so provide it before anything imports jax/concourse.
if "antenv.axon_hooks" not in sys.modules:
    _ah = types.ModuleType("antenv.axon_hooks")
    _ah._hook = None

    def _set_hook(hook):
        _ah._hook = hook

    def _get_hook():
        return _ah._hook

    _ah.set_axon_ntff_profile_hook = _set_hook
    _ah.get_axon_ntff_profile_hook = _get_hook
    sys.modules["antenv.axon_hooks"] = _ah

import ml_dtypes  # noqa: E402
import concourse.bass as bass  # noqa: E402
import concourse.bacc as bacc  # noqa: E402
import concourse.tile as tile  # noqa: E402
from contextlib import ExitStack  # noqa: E402
from concourse import mybir  # noqa: E402
from concourse import bass_utils  # noqa: E402
from concourse.masks import make_identity  # noqa: E402

H = 0.5
EPS = 1e-8
B, D, R, O = 4096, 256, 128, 64
NCORES = 8
BL = B // NCORES          # 512 batch rows per core
NT = BL // 128            # 4 partition tiles per core
RO = R * O                # 8192 G columns per row
F32 = mybir.dt.float32
BF16 = mybir.dt.bfloat16
EXPF = mybir.ActivationFunctionType.Exp
ADD = mybir.AluOpType.add

# const blob column offsets (bf16 elements)
OBM, OK2, OW2, OXT = 0, 256, 384, 520
NC_BASE = OXT + 1024          # 1544
OA = NC_BASE                  # A appended when sigmas are non-uniform

_CACHE = {}
LAST_RESULT = None
TRACE = False
TRACE_DIR = "/root/problem/work/trace_out"
# Level-1 tree via SDMA accum_op=add was tried and wedges the device: the
# SWDGE descriptor generator (GpSimd) deadlocks against DVE's shared SBUF
# port (exclusive lock). Keep the level-1 add on DVE.
L1_DMA = os.environ.get("L1_DMA", "0") == "1"


def _build(use_a: bool):
    ncols = NC_BASE + (256 if use_a else 0)
    nc = bacc.Bacc("TRN2", target_bir_lowering=False, debug=False)
    CONST = nc.dram_tensor("CONST", [128, ncols], BF16, kind="ExternalInput")
    WT = nc.dram_tensor("WT", [D, RO], BF16, kind="ExternalInput")
    out = nc.dram_tensor("out", [BL, O], F32, kind="ExternalOutput")

    with tile.TileContext(nc) as tc, ExitStack() as ctx:
        consts = ctx.enter_context(tc.tile_pool(name="consts", bufs=1))
        work = ctx.enter_context(tc.tile_pool(name="work", bufs=2))
        gsbp = ctx.enter_context(tc.tile_pool(name="gsb", bufs=2))
        gap = ctx.enter_context(tc.tile_pool(name="ga", bufs=2))
        gbp = ctx.enter_context(tc.tile_pool(name="gb", bufs=2))
        treep = ctx.enter_context(tc.tile_pool(name="tree", bufs=2))

        # ---- packed const DMA split across both HWDGE queues; Wt streams
        # on sync+gpsimd behind it ----
        conc = consts.tile([128, ncols], BF16, tag="conc")
        half = (ncols // 2) & ~3
        nc.sync.dma_start(out=conc[:, 0:half], in_=CONST[:, 0:half])
        nc.scalar.dma_start(out=conc[:, half:], in_=CONST[:, half:])
        bm_sb = conc[:, OBM:OBM + 256].rearrange("p (c r) -> p c r", r=R)
        k2_sb = conc[0:1, OK2:OK2 + R]
        w2b_sb = conc[:, OW2:OW2 + O + 1]          # [R, 65]: W2+b ++ ones col
        xTv = conc[:, OXT:OXT + 1024].rearrange("p (c j) -> p c j", j=BL)
        if use_a:
            a_sb = conc[:, OA:OA + 256].rearrange("p (c r) -> p c r", r=R)
        identB = consts.tile([128, 128], BF16, tag="idb")
        make_identity(nc, identB)
        ones_sb = consts.tile([1, 128], BF16, tag="ones")
        nc.vector.memset(ones_sb, 1.0)
        # Wt pair-chunk tiles: c0 on sync ring, c1 on gpsimd (SWDGE) ring
        wt_sb = [[None] * 4, [None] * 4]
        for q in range(4):
            for c in range(2):
                t_ = consts.tile([128, 2048], BF16, tag=f"wt{c}{q}")
                eng = nc.sync if c == 0 else nc.gpsimd
                eng.dma_start(out=t_[:, :],
                              in_=WT[c * 128:(c + 1) * 128,
                                     q * 2048:(q + 1) * 2048])
                wt_sb[c][q] = t_
        if use_a:
            x2T = consts.tile([128, 2, BL], BF16, tag="x2T")
            for c in range(2):
                nc.scalar.square(x2T[:, c, :], xTv[:, c, :])

        # per-tile softmax state, alive through phase 2
        e_bf = [consts.tile([128, R], BF16, tag=f"e{t}", name=f"e_{t}")
                for t in range(NT)]
        rs_all = consts.tile([128, NT], F32, tag="rs")
        o2_sb = consts.tile([128, NT * (O + 1)], F32, tag="o2sb")

        # ---- phase 1: logits / exp / e^T / out2 for all tiles ----
        with tc.tile_pool(name="ps_pre", bufs=1, space="PSUM") as ps_pre, \
             tc.tile_pool(name="ps_eT", bufs=2, space="PSUM") as ps_eT:
            m_ps = [ps_pre.tile([128, R], F32, tag=f"m{t}", name=f"m_{t}")
                    for t in range(NT)]                           # 1 bank each
            for t in range(NT):
                bs = slice(t * 128, (t + 1) * 128)
                mt = m_ps[t]
                first = True
                if use_a:
                    for c in range(2):
                        nc.tensor.matmul(mt, lhsT=x2T[:, c, bs],
                                         rhs=a_sb[:, c, :],
                                         start=first, stop=False)
                        first = False
                for c in range(2):
                    nc.tensor.matmul(mt, lhsT=xTv[:, c, bs], rhs=bm_sb[:, c, :],
                                     start=first, stop=False)
                    first = False
                nc.tensor.matmul(mt, lhsT=ones_sb, rhs=k2_sb,
                                 start=False, stop=True)
            for t in range(NT):
                nc.scalar.activation(e_bf[t], m_ps[t], EXPF, bias=0.0, scale=1.0)
                eT_ps = ps_eT.tile([128, 128], BF16, tag="eT")
                nc.tensor.transpose(eT_ps, e_bf[t], identB)
                eT_sb = work.tile([128, 128], BF16, tag="eTsb")
                nc.vector.tensor_copy(eT_sb, eT_ps)
                # out2 reuses tile t's freed m bank (m was consumed by exp)
                nc.tensor.matmul(m_ps[t][:, 0:O + 1], lhsT=eT_sb,
                                 rhs=w2b_sb, start=True, stop=True)
                nc.vector.tensor_copy(o2_sb[:, t * 65:(t + 1) * 65],
                                      m_ps[t][:, 0:O + 1])
                nc.vector.reciprocal(rs_all[:, t:t + 1],
                                     o2_sb[:, t * 65 + O:t * 65 + O + 1])
                # pre-normalize: e <- e/sum_e and out2 <- out2/sum_e, so the
                # per-tile epilogue is a plain add (no tensor_scalar_mul)
                nc.scalar.mul(e_bf[t], e_bf[t], rs_all[:, t:t + 1])
                nc.vector.tensor_scalar_mul(
                    o2_sb[:, t * 65:t * 65 + O], o2_sb[:, t * 65:t * 65 + O],
                    rs_all[:, t:t + 1])


        # ---- phase 2: G matmuls, drains, multiplies, tree ----
        # Emission is software-pipelined: tile t's tree/epilogue instructions
        # are emitted AFTER tile t+1's drains/multiplies so the DVE FIFO never
        # blocks the next tile's multiplies behind a finished tile's tree.
        def reduce_half(z3, t, obase, n, sub):
            # z3: [128, n, 64] view whose o-axis starts at absolute o=obase
            t2 = treep.tile([128, n, 32], BF16, tag=f"t2{sub}",
                            name=f"t2_{t}{sub}")
            nc.vector.tensor_add(t2, z3[:, :, 0:32], z3[:, :, 32:64])
            t3 = treep.tile([128, n, 16], BF16, tag=f"t3{sub}",
                            name=f"t3_{t}{sub}")
            nc.vector.tensor_add(t3, t2[:, :, 0:16], t2[:, :, 16:32])
            t4 = treep.tile([128, n, 8], BF16, tag=f"t4{sub}",
                            name=f"t4_{t}{sub}")
            nc.vector.tensor_add(t4, t3[:, :, 0:8], t3[:, :, 8:16])
            t5 = treep.tile([128, n, 4], BF16, tag=f"t5{sub}",
                            name=f"t5_{t}{sub}")
            nc.vector.tensor_add(t5, t4[:, :, 0:4], t4[:, :, 4:8])
            t6 = treep.tile([128, n, 2], BF16, tag=f"t6{sub}",
                            name=f"t6_{t}{sub}")
            nc.vector.tensor_add(t6, t5[:, :, 0:2], t5[:, :, 2:4])
            red = work.tile([128, n, 1], BF16, tag=f"red{sub}",
                            name=f"red_{t}{sub}")
            nc.vector.tensor_add(red, t6[:, :, 0:1], t6[:, :, 1:2])
            osb = work.tile([128, n], F32, tag=f"osb{sub}", name=f"osb_{t}{sub}")
            nc.vector.tensor_add(osb, red.rearrange("p o () -> p o"),
                                 o2_sb[:, t * 65 + obase:t * 65 + obase + n])
            nc.sync.dma_start(out=out[t * 128:(t + 1) * 128, obase:obase + n],
                              in_=osb)

        with tc.tile_pool(name="ps_g", bufs=2, space="PSUM") as ps_g:
            state = {}

            def emit_G(t):
                bs = slice(t * 128, (t + 1) * 128)
                last = t == NT - 1
                gsb = gsbp.tile([128, RO], BF16, tag="gsb", name=f"gsb_{t}")
                ga = gap.tile([128, 4096], BF16, tag="ga", name=f"ga_{t}")
                gb = gbp.tile([128, 4096], BF16, tag="gb", name=f"gb_{t}")
                ga3 = ga.rearrange("p (o r) -> p o r", r=64)
                gb3 = gb.rearrange("p (o r) -> p o r", r=64)
                gsb3 = gsb.rearrange("p (o r) -> p o r", r=64)
                # last tile: o-low pair-chunks first so its first half-tree
                # can start while the o-high matmuls still run
                order = (0, 2, 1, 3) if last else (0, 1, 2, 3)
                for i, pq in enumerate(order):
                    gt = ps_g.tile([128, 2048], F32, tag="g", name=f"g_{t}_{pq}")
                    for c in range(2):
                        for h in range(4):
                            nc.tensor.matmul(
                                gt[:, h * 512:(h + 1) * 512],
                                lhsT=xTv[:, c, bs],
                                rhs=wt_sb[c][pq][:, h * 512:(h + 1) * 512],
                                start=(c == 0), stop=(c == 1),
                            )
                    half, oq = divmod(pq, 2)
                    dst3 = (ga3 if half == 0 else gb3)[:, oq * 32:(oq + 1) * 32, :]
                    ebc = (e_bf[t][:, half * 64:(half + 1) * 64]
                           .rearrange("p r -> p () r").broadcast_to((128, 32, 64)))
                    if last and i == 3:
                        # final chunk of the run: multiply straight from PSUM
                        # (1x) to skip the ScalarE drain on the critical tail
                        nc.vector.tensor_mul(
                            dst3,
                            gt.rearrange("p (o r) -> p o r", r=64), ebc)
                    else:
                        nc.scalar.copy(gsb[:, pq * 2048:(pq + 1) * 2048], gt)
                        nc.vector.tensor_mul(
                            dst3, gsb3[:, pq * 32:(pq + 1) * 32, :], ebc)
                    if last and i == 1:
                        # o 0:32 complete in both halves: first half-tree now
                        t1a = treep.tile([128, 32, 64], BF16, tag="t1a")
                        nc.vector.tensor_add(t1a, ga3[:, 0:32, :],
                                             gb3[:, 0:32, :])
                        reduce_half(t1a, t, 0, 32, "a")
                state["ga"], state["gb"] = ga3, gb3

            def emit_tree(t):
                ga3, gb3 = state["t%d" % t]
                if L1_DMA:
                    nc.gpsimd.dma_start(
                        out=ga3.rearrange("p o r -> p (o r)"),
                        in_=gb3.rearrange("p o r -> p (o r)"), accum_op=ADD)
                    z = ga3
                else:
                    t1 = treep.tile([128, 64, 64], BF16, tag="t1",
                                    name=f"t1_{t}")
                    nc.vector.tensor_add(t1, ga3, gb3)
                    z = t1
                reduce_half(z, t, 0, 64, "")  # full width: obase=0, n=64

            for t in range(NT):
                emit_G(t)
                state["t%d" % t] = (state["ga"], state["gb"])
                if t > 0:
                    emit_tree(t - 1)
            # last tile: second half-tree only (first half emitted inline)
            ga3, gb3 = state["t%d" % (NT - 1)]
            t1b = treep.tile([128, 32, 64], BF16, tag="t1b")
            nc.vector.tensor_add(t1b, ga3[:, 32:64, :], gb3[:, 32:64, :])
            reduce_half(t1b, NT - 1, 32, 32, "b")

    nc.finalize()
    return nc


def _get_nc(use_a: bool):
    key = ("nc", use_a)
    if key not in _CACHE:
        _CACHE[key] = _build(use_a)
    return _CACHE[key]


def _host_prep(centers, sigmas, W, b):
    c64 = centers.astype(np.float64)
    S = (H / sigmas.astype(np.float64) ** 2) + EPS          # (D,R)
    use_a = not np.allclose(S, S.flat[0])
    bf = ml_dtypes.bfloat16
    ncols = NC_BASE + (256 if use_a else 0)
    CB = np.zeros((128, ncols), dtype=bf)
    Bm = (2.0 * S * c64 / D).astype(bf)                      # X coeff
    CB[:, OBM:OBM + 128] = Bm[0:128]
    CB[:, OBM + 128:OBM + 256] = Bm[128:256]
    K2 = (-(S * c64 * c64).sum(axis=0) / D).astype(bf)
    CB[0, OK2:OK2 + R] = K2
    W2b = np.concatenate(
        [W[D * R:].astype(np.float64) + b[None, :].astype(np.float64),
         np.ones((R, 1))], axis=1
    ).astype(bf)
    CB[:, OW2:OW2 + O + 1] = W2b
    if use_a:
        A = (-S / D).astype(bf)
        CB[:, OA:OA + 128] = A[0:128]
        CB[:, OA + 128:OA + 256] = A[128:256]
    W1 = W[: D * R].reshape(2, 64, D, O)          # (half, rr, d, o)
    # split-half o-major: Wt[d, h*4096 + o*64 + rr] = W1[h, rr, d, o]
    Wt = np.ascontiguousarray(
        W1.transpose(2, 0, 3, 1).reshape(D, RO)).astype(bf)
    return use_a, CB, Wt


def kernel(X, centers, sigmas, W, b):
    global LAST_RESULT
    X = np.asarray(X, dtype=np.float32)
    centers = np.asarray(centers, dtype=np.float32)
    sigmas = np.asarray(sigmas, dtype=np.float32)
    W = np.asarray(W, dtype=np.float32)
    b = np.asarray(b, dtype=np.float32)

    use_a, CB, Wt = _host_prep(centers, sigmas, W, b)
    Xb = X.astype(ml_dtypes.bfloat16)
    nc = _get_nc(use_a)
    in_maps = []
    for k in range(NCORES):
        cb = CB.copy()
        xt = Xb[k * BL:(k + 1) * BL].T                      # (D, BL)
        cb[:, OXT:OXT + BL] = xt[0:128]
        cb[:, OXT + BL:OXT + 1024] = xt[128:256]
        in_maps.append({"CONST": cb, "WT": Wt})
    kw = {}
    if TRACE:
        import shutil
        shutil.rmtree(TRACE_DIR, ignore_errors=True)
        kw = {"trace": True, "tmpdir": TRACE_DIR}
    res = bass_utils.run_bass_kernel_spmd(
        nc, in_maps, core_ids=list(range(NCORES)), **kw
    )
    LAST_RESULT = res
    return np.concatenate([res.results[k]["out"] for k in range(NCORES)], axis=0)


# revision 29
# speedup vs baseline: 1.0210x; 1.0087x over previous
"""HTSK fuzzy-system kernel for Trainium2 (Bass/Tile), 8-core data-parallel.

Math (per batch row b):
  S     = H/sigma^2 + EPS                          (D,R)
  m     = mean_d(-(X_bd - C_dr)^2 * S_dr)          (B,R)
        = X^2 @ (-S/D) + X @ (2*S*C/D) + K2        (matmul expansion)
  e     = exp(m)            (unnormalized softmax; m is bounded, no max needed)
  out   = (1/sum_r e) * ( sum_r e_br * G_bro  +  e @ (W2 + 1 b^T) )
  G     = X @ Wt,  Wt[d, h*4096 + o*64 + rr] = W[(h*64+rr)*D+d, o]

G layout: r split in low/high halves (h), o-major within each, rr innermost.
- innermost rr is step-1 so the e-broadcast multiply runs in DVE 2x_1P mode
- the r-halves live in two separate contiguous 4KB-per-partition tiles, so
  tree level 1 is ONE contiguous SBUF->SBUF DMA with the CCE inline adder
  (accum_op=add), running on the otherwise-idle DMA engines
When sigmas are uniform the X^2 term is constant over r and cancels in the
softmax, so the A-matmuls are dropped. sum_r e comes from a ones column
appended to W2.

Two phases per core:
  1) logits/exp/e-transpose/out2 for all 4 row-tiles, using 3 PSUM banks
     (scoped pools), overlapped with the Wt DMA stream
  2) G phase: all 8 PSUM banks as 2x[128,2048] fp32 ping-pong; per
     pair-chunk 8 matmuls (stationary changes once), ScalarE drain to bf16,
     DVE 2x multiply; tree L1 on DMA (CCE), L2..L7 + epilogue on DVE

Sharding: batch B=4096 split 512 rows per core; weights replicated.
All small constants + X^T ship in one packed [128, x] DMA blob per core.
"""
import os
import sys
import types

import numpy as np

sys.path.insert(0, "/opt/trn_rl_repo")

# NTFF profile-hook registry: trn_boot sets it at jax init, concourse
# bass_utils reads it when trace=True. The container's antenv package lacks
# this submodule, so provide it before anything imports jax/concourse.
if "antenv.axon_hooks" not in sys.modules:
    _ah = types.ModuleType("antenv.axon_hooks")
    _ah._hook = None

    def _set_hook(hook):
        _ah._hook = hook

    def _get_hook():
        return _ah._hook

    _ah.set_axon_ntff_profile_hook = _set_hook
    _ah.get_axon_ntff_profile_hook = _get_hook
    sys.modules["antenv.axon_hooks"] = _ah

import ml_dtypes  # noqa: E402
import concourse.bass as bass  # noqa: E402
import concourse.bacc as bacc  # noqa: E402
import concourse.tile as tile  # noqa: E402
from contextlib import ExitStack  # noqa: E402
from concourse import mybir  # noqa: E402
from concourse import bass_utils  # noqa: E402
from concourse.masks import make_identity  # noqa: E402

H = 0.5
EPS = 1e-8
B, D, R, O = 4096, 256, 128, 64
NCORES = 8
BL = B // NCORES          # 512 batch rows per core
NT = BL // 128            # 4 partition tiles per core
RO = R * O                # 8192 G columns per row
F32 = mybir.dt.float32
BF16 = mybir.dt.bfloat16
EXPF = mybir.ActivationFunctionType.Exp
ADD = mybir.AluOpType.add

# const blob column offsets (bf16 elements)
OBM, OK2, OW2, OXT = 0, 256, 384, 520
NC_BASE = OXT + 1024          # 1544
OA = NC_BASE                  # A appended when sigmas are non-uniform

_CACHE = {}
LAST_RESULT = None
TRACE = False
TRACE_DIR = "/root/problem/work/trace_out"
# Level-1 tree via SDMA accum_op=add was tried and wedges the device: the
# SWDGE descriptor generator (GpSimd) deadlocks against DVE's shared SBUF
# port (exclusive lock). Keep the level-1 add on DVE.
L1_DMA = os.environ.get("L1_DMA", "0") == "1"


def _build(use_a: bool):
    ncols = NC_BASE + (256 if use_a else 0)
    nc = bacc.Bacc("TRN2", target_bir_lowering=False, debug=False)
    CONST = nc.dram_tensor("CONST", [128, ncols], BF16, kind="ExternalInput")
    WT = nc.dram_tensor("WT", [D, RO], BF16, kind="ExternalInput")
    out = nc.dram_tensor("out", [BL, O], F32, kind="ExternalOutput")

    with tile.TileContext(nc) as tc, ExitStack() as ctx:
        consts = ctx.enter_context(tc.tile_pool(name="consts", bufs=1))
        work = ctx.enter_context(tc.tile_pool(name="work", bufs=2))
        gsbp = ctx.enter_context(tc.tile_pool(name="gsb", bufs=2))
        gap = ctx.enter_context(tc.tile_pool(name="ga", bufs=2))
        gbp = ctx.enter_context(tc.tile_pool(name="gb", bufs=2))
        treep = ctx.enter_context(tc.tile_pool(name="tree", bufs=2))

        # ---- packed const DMA split across both HWDGE queues; Wt streams
        # on sync+gpsimd behind it ----
        conc = consts.tile([128, ncols], BF16, tag="conc")
        half = (ncols // 2) & ~3
        nc.sync.dma_start(out=conc[:, 0:half], in_=CONST[:, 0:half])
        nc.scalar.dma_start(out=conc[:, half:], in_=CONST[:, half:])
        bm_sb = conc[:, OBM:OBM + 256].rearrange("p (c r) -> p c r", r=R)
        k2_sb = conc[0:1, OK2:OK2 + R]
        w2b_sb = conc[:, OW2:OW2 + O + 1]          # [R, 65]: W2+b ++ ones col
        xTv = conc[:, OXT:OXT + 1024].rearrange("p (c j) -> p c j", j=BL)
        if use_a:
            a_sb = conc[:, OA:OA + 256].rearrange("p (c r) -> p c r", r=R)
        identB = consts.tile([128, 128], BF16, tag="idb")
        make_identity(nc, identB)
        ones_sb = consts.tile([1, 128], BF16, tag="ones")
        nc.vector.memset(ones_sb, 1.0)
        # Wt pair-chunk tiles: c0 on sync ring, c1 on gpsimd (SWDGE) ring
        wt_sb = [[None] * 4, [None] * 4]
        for q in range(4):
            for c in range(2):
                t_ = consts.tile([128, 2048], BF16, tag=f"wt{c}{q}")
                eng = nc.sync if c == 0 else nc.gpsimd
                eng.dma_start(out=t_[:, :],
                              in_=WT[c * 128:(c + 1) * 128,
                                     q * 2048:(q + 1) * 2048])
                wt_sb[c][q] = t_
        if use_a:
            x2T = consts.tile([128, 2, BL], BF16, tag="x2T")
            for c in range(2):
                nc.scalar.square(x2T[:, c, :], xTv[:, c, :])

        # per-tile softmax state, alive through phase 2
        e_bf = [consts.tile([128, R], BF16, tag=f"e{t}", name=f"e_{t}")
                for t in range(NT)]
        rs_all = consts.tile([128, NT], F32, tag="rs")
        o2_sb = consts.tile([128, NT * (O + 1)], F32, tag="o2sb")

        # ---- phase 1: logits / exp / e^T / out2 for all tiles ----
        with tc.tile_pool(name="ps_pre", bufs=1, space="PSUM") as ps_pre, \
             tc.tile_pool(name="ps_eT", bufs=2, space="PSUM") as ps_eT:
            m_ps = [ps_pre.tile([128, R], F32, tag=f"m{t}", name=f"m_{t}")
                    for t in range(NT)]                           # 1 bank each
            for t in range(NT):
                bs = slice(t * 128, (t + 1) * 128)
                mt = m_ps[t]
                first = True
                if use_a:
                    for c in range(2):
                        nc.tensor.matmul(mt, lhsT=x2T[:, c, bs],
                                         rhs=a_sb[:, c, :],
                                         start=first, stop=False)
                        first = False
                for c in range(2):
                    nc.tensor.matmul(mt, lhsT=xTv[:, c, bs], rhs=bm_sb[:, c, :],
                                     start=first, stop=False)
                    first = False
                nc.tensor.matmul(mt, lhsT=ones_sb, rhs=k2_sb,
                                 start=False, stop=True)
            for t in range(NT):
                nc.scalar.activation(e_bf[t], m_ps[t], EXPF, bias=0.0, scale=1.0)
                eT_ps = ps_eT.tile([128, 128], BF16, tag="eT")
                nc.tensor.transpose(eT_ps, e_bf[t], identB)
                eT_sb = work.tile([128, 128], BF16, tag="eTsb")
                nc.vector.tensor_copy(eT_sb, eT_ps)
                # out2 reuses tile t's freed m bank (m was consumed by exp)
                nc.tensor.matmul(m_ps[t][:, 0:O + 1], lhsT=eT_sb,
                                 rhs=w2b_sb, start=True, stop=True)
                nc.vector.tensor_copy(o2_sb[:, t * 65:(t + 1) * 65],
                                      m_ps[t][:, 0:O + 1])
                nc.vector.reciprocal(rs_all[:, t:t + 1],
                                     o2_sb[:, t * 65 + O:t * 65 + O + 1])
                # pre-normalize: e <- e/sum_e and out2 <- out2/sum_e, so the
                # per-tile epilogue is a plain add (no tensor_scalar_mul)
                nc.scalar.mul(e_bf[t], e_bf[t], rs_all[:, t:t + 1])
                nc.vector.tensor_scalar_mul(
                    o2_sb[:, t * 65:t * 65 + O], o2_sb[:, t * 65:t * 65 + O],
                    rs_all[:, t:t + 1])


        # ---- phase 2: G matmuls, drains, multiplies, tree ----
        # Emission is software-pipelined: tile t's tree/epilogue instructions
        # are emitted AFTER tile t+1's drains/multiplies so the DVE FIFO never
        # blocks the next tile's multiplies behind a finished tile's tree.
        def reduce_half(z3, t, obase, n, sub):
            # z3: [128, n, 64] view whose o-axis starts at absolute o=obase
            t2 = treep.tile([128, n, 32], BF16, tag=f"t2{sub}",
                            name=f"t2_{t}{sub}")
            nc.vector.tensor_add(t2, z3[:, :, 0:32], z3[:, :, 32:64])
            t3 = treep.tile([128, n, 16], BF16, tag=f"t3{sub}",
                            name=f"t3_{t}{sub}")
            nc.vector.tensor_add(t3, t2[:, :, 0:16], t2[:, :, 16:32])
            t4 = treep.tile([128, n, 8], BF16, tag=f"t4{sub}",
                            name=f"t4_{t}{sub}")
            nc.vector.tensor_add(t4, t3[:, :, 0:8], t3[:, :, 8:16])
            t5 = treep.tile([128, n, 4], BF16, tag=f"t5{sub}",
                            name=f"t5_{t}{sub}")
            nc.vector.tensor_add(t5, t4[:, :, 0:4], t4[:, :, 4:8])
            t6 = treep.tile([128, n, 2], BF16, tag=f"t6{sub}",
                            name=f"t6_{t}{sub}")
            nc.vector.tensor_add(t6, t5[:, :, 0:2], t5[:, :, 2:4])
            red = work.tile([128, n, 1], BF16, tag=f"red{sub}",
                            name=f"red_{t}{sub}")
            nc.vector.tensor_add(red, t6[:, :, 0:1], t6[:, :, 1:2])
            osb = work.tile([128, n], F32, tag=f"osb{sub}", name=f"osb_{t}{sub}")
            nc.vector.tensor_add(osb, red.rearrange("p o () -> p o"),
                                 o2_sb[:, t * 65 + obase:t * 65 + obase + n])
            nc.sync.dma_start(out=out[t * 128:(t + 1) * 128, obase:obase + n],
                              in_=osb)

        with tc.tile_pool(name="ps_g", bufs=2, space="PSUM") as ps_g:
            from collections import deque
            state = {}
            pending = deque()   # generators of deferred tree steps

            def pump(k):
                # emit up to k deferred tree ops into the DVE stream
                steps = 0
                while pending and steps < k:
                    try:
                        next(pending[0])
                        steps += 1
                    except StopIteration:
                        pending.popleft()

            def reduce_steps(z3, t, obase, n, sub):
                # generator form of reduce_half: one DVE op per step
                t2 = treep.tile([128, n, 32], BF16, tag=f"t2{sub}",
                                name=f"t2g_{t}{sub}")
                nc.vector.tensor_add(t2, z3[:, :, 0:32], z3[:, :, 32:64])
                yield
                t3 = treep.tile([128, n, 16], BF16, tag=f"t3{sub}",
                                name=f"t3g_{t}{sub}")
                nc.vector.tensor_add(t3, t2[:, :, 0:16], t2[:, :, 16:32])
                yield
                t4 = treep.tile([128, n, 8], BF16, tag=f"t4{sub}",
                                name=f"t4g_{t}{sub}")
                nc.vector.tensor_add(t4, t3[:, :, 0:8], t3[:, :, 8:16])
                yield
                t5 = treep.tile([128, n, 4], BF16, tag=f"t5{sub}",
                                name=f"t5g_{t}{sub}")
                nc.vector.tensor_add(t5, t4[:, :, 0:4], t4[:, :, 4:8])
                yield
                t6 = treep.tile([128, n, 2], BF16, tag=f"t6{sub}",
                                name=f"t6g_{t}{sub}")
                nc.vector.tensor_add(t6, t5[:, :, 0:2], t5[:, :, 2:4])
                yield
                red = work.tile([128, n, 1], BF16, tag=f"red{sub}",
                                name=f"redg_{t}{sub}")
                nc.vector.tensor_add(red, t6[:, :, 0:1], t6[:, :, 1:2])
                yield
                osb = work.tile([128, n], F32, tag=f"osb{sub}",
                                name=f"osbg_{t}{sub}")
                nc.vector.tensor_add(osb, red.rearrange("p o () -> p o"),
                                     o2_sb[:, t * 65 + obase:t * 65 + obase + n])
                nc.sync.dma_start(
                    out=out[t * 128:(t + 1) * 128, obase:obase + n], in_=osb)
                yield

            def tree_steps(t):
                ga3, gb3 = state["t%d" % t]
                t1 = treep.tile([128, 64, 64], BF16, tag="t1", name=f"t1_{t}")
                nc.vector.tensor_add(t1, ga3, gb3)
                yield
                yield from reduce_steps(t1, t, 0, 64, "")

            def emit_G(t):
                bs = slice(t * 128, (t + 1) * 128)
                last = t == NT - 1
                gsb = gsbp.tile([128, RO], BF16, tag="gsb", name=f"gsb_{t}")
                ga = gap.tile([128, 4096], BF16, tag="ga", name=f"ga_{t}")
                gb = gbp.tile([128, 4096], BF16, tag="gb", name=f"gb_{t}")
                ga3 = ga.rearrange("p (o r) -> p o r", r=64)
                gb3 = gb.rearrange("p (o r) -> p o r", r=64)
                gsb3 = gsb.rearrange("p (o r) -> p o r", r=64)
                # last tile: o-low pair-chunks first so its first half-tree
                # can start while the o-high matmuls still run
                order = (0, 2, 1, 3) if last else (0, 1, 2, 3)
                for i, pq in enumerate(order):
                    gt = ps_g.tile([128, 2048], F32, tag="g", name=f"g_{t}_{pq}")
                    for c in range(2):
                        for h in range(4):
                            nc.tensor.matmul(
                                gt[:, h * 512:(h + 1) * 512],
                                lhsT=xTv[:, c, bs],
                                rhs=wt_sb[c][pq][:, h * 512:(h + 1) * 512],
                                start=(c == 0), stop=(c == 1),
                            )
                    half, oq = divmod(pq, 2)
                    dst3 = (ga3 if half == 0 else gb3)[:, oq * 32:(oq + 1) * 32, :]
                    ebc = (e_bf[t][:, half * 64:(half + 1) * 64]
                           .rearrange("p r -> p () r").broadcast_to((128, 32, 64)))
                    if last and i == 3:
                        # final chunk of the run: multiply straight from PSUM
                        # (1x) to skip the ScalarE drain on the critical tail
                        nc.vector.tensor_mul(
                            dst3,
                            gt.rearrange("p (o r) -> p o r", r=64), ebc)
                    else:
                        nc.scalar.copy(gsb[:, pq * 2048:(pq + 1) * 2048], gt)
                        nc.vector.tensor_mul(
                            dst3, gsb3[:, pq * 32:(pq + 1) * 32, :], ebc)
                    # interleave up to 3 deferred tree ops of the previous
                    # tile into the DVE stream after each chunk's multiply
                    pump(3)
                    if last and i == 1:
                        # o 0:32 complete in both halves: first half-tree now
                        t1a = treep.tile([128, 32, 64], BF16, tag="t1a")
                        nc.vector.tensor_add(t1a, ga3[:, 0:32, :],
                                             gb3[:, 0:32, :])
                        reduce_half(t1a, t, 0, 32, "a")
                state["ga"], state["gb"] = ga3, gb3

            for t in range(NT):
                if t > 0:
                    pending.append(tree_steps(t - 1))
                emit_G(t)
                state["t%d" % t] = (state["ga"], state["gb"])
            pump(10 ** 9)   # flush any remaining deferred steps
            # last tile: second half-tree only (first half emitted inline)
            ga3, gb3 = state["t%d" % (NT - 1)]
            t1b = treep.tile([128, 32, 64], BF16, tag="t1b")
            nc.vector.tensor_add(t1b, ga3[:, 32:64, :], gb3[:, 32:64, :])
            reduce_half(t1b, NT - 1, 32, 32, "b")

    nc.finalize()
    return nc


def _get_nc(use_a: bool):
    key = ("nc", use_a)
    if key not in _CACHE:
        _CACHE[key] = _build(use_a)
    return _CACHE[key]


def _host_prep(centers, sigmas, W, b):
    c64 = centers.astype(np.float64)
    S = (H / sigmas.astype(np.float64) ** 2) + EPS          # (D,R)
    use_a = not np.allclose(S, S.flat[0])
    bf = ml_dtypes.bfloat16
    ncols = NC_BASE + (256 if use_a else 0)
    CB = np.zeros((128, ncols), dtype=bf)
    Bm = (2.0 * S * c64 / D).astype(bf)                      # X coeff
    CB[:, OBM:OBM + 128] = Bm[0:128]
    CB[:, OBM + 128:OBM + 256] = Bm[128:256]
    K2 = (-(S * c64 * c64).sum(axis=0) / D).astype(bf)
    CB[0, OK2:OK2 + R] = K2
    W2b = np.concatenate(
        [W[D * R:].astype(np.float64) + b[None, :].astype(np.float64),
         np.ones((R, 1))], axis=1
    ).astype(bf)
    CB[:, OW2:OW2 + O + 1] = W2b
    if use_a:
        A = (-S / D).astype(bf)
        CB[:, OA:OA + 128] = A[0:128]
        CB[:, OA + 128:OA + 256] = A[128:256]
    W1 = W[: D * R].reshape(2, 64, D, O)          # (half, rr, d, o)
    # split-half o-major: Wt[d, h*4096 + o*64 + rr] = W1[h, rr, d, o]
    Wt = np.ascontiguousarray(
        W1.transpose(2, 0, 3, 1).reshape(D, RO)).astype(bf)
    return use_a, CB, Wt


def kernel(X, centers, sigmas, W, b):
    global LAST_RESULT
    X = np.asarray(X, dtype=np.float32)
    centers = np.asarray(centers, dtype=np.float32)
    sigmas = np.asarray(sigmas, dtype=np.float32)
    W = np.asarray(W, dtype=np.float32)
    b = np.asarray(b, dtype=np.float32)

    use_a, CB, Wt = _host_prep(centers, sigmas, W, b)
    Xb = X.astype(ml_dtypes.bfloat16)
    nc = _get_nc(use_a)
    in_maps = []
    for k in range(NCORES):
        cb = CB.copy()
        xt = Xb[k * BL:(k + 1) * BL].T                      # (D, BL)
        cb[:, OXT:OXT + BL] = xt[0:128]
        cb[:, OXT + BL:OXT + 1024] = xt[128:256]
        in_maps.append({"CONST": cb, "WT": Wt})
    kw = {}
    if TRACE:
        import shutil
        shutil.rmtree(TRACE_DIR, ignore_errors=True)
        kw = {"trace": True, "tmpdir": TRACE_DIR}
    res = bass_utils.run_bass_kernel_spmd(
        nc, in_maps, core_ids=list(range(NCORES)), **kw
    )
    LAST_RESULT = res
    return np.concatenate([res.results[k]["out"] for k in range(NCORES)], axis=0)


# revision 31
# speedup vs baseline: 1.0214x; 1.0003x over previous
"""HTSK fuzzy-system kernel for Trainium2 (Bass/Tile), 8-core data-parallel.

Math (per batch row b):
  S     = H/sigma^2 + EPS                          (D,R)
  m     = mean_d(-(X_bd - C_dr)^2 * S_dr)          (B,R)
        = X^2 @ (-S/D) + X @ (2*S*C/D) + K2        (matmul expansion)
  e     = exp(m)            (unnormalized softmax; m is bounded, no max needed)
  out   = (1/sum_r e) * ( sum_r e_br * G_bro  +  e @ (W2 + 1 b^T) )
  G     = X @ Wt,  Wt[d, h*4096 + o*64 + rr] = W[(h*64+rr)*D+d, o]

G layout: r split in low/high halves (h), o-major within each, rr innermost.
- innermost rr is step-1 so the e-broadcast multiply runs in DVE 2x_1P mode
- the r-halves live in two separate contiguous 4KB-per-partition tiles, so
  tree level 1 is ONE contiguous SBUF->SBUF DMA with the CCE inline adder
  (accum_op=add), running on the otherwise-idle DMA engines
When sigmas are uniform the X^2 term is constant over r and cancels in the
softmax, so the A-matmuls are dropped. sum_r e comes from a ones column
appended to W2.

Two phases per core:
  1) logits/exp/e-transpose/out2 for all 4 row-tiles, using 3 PSUM banks
     (scoped pools), overlapped with the Wt DMA stream
  2) G phase: all 8 PSUM banks as 2x[128,2048] fp32 ping-pong; per
     pair-chunk 8 matmuls (stationary changes once), ScalarE drain to bf16,
     DVE 2x multiply; tree L1 on DMA (CCE), L2..L7 + epilogue on DVE

Sharding: batch B=4096 split 512 rows per core; weights replicated.
All small constants + X^T ship in one packed [128, x] DMA blob per core.
"""
import os
import sys
import types

import numpy as np

sys.path.insert(0, "/opt/trn_rl_repo")

# NTFF profile-hook registry: trn_boot sets it at jax init, concourse
# bass_utils reads it when trace=True. The container's antenv package lacks
# this submodule, so provide it before anything imports jax/concourse.
if "antenv.axon_hooks" not in sys.modules:
    _ah = types.ModuleType("antenv.axon_hooks")
    _ah._hook = None

    def _set_hook(hook):
        _ah._hook = hook

    def _get_hook():
        return _ah._hook

    _ah.set_axon_ntff_profile_hook = _set_hook
    _ah.get_axon_ntff_profile_hook = _get_hook
    sys.modules["antenv.axon_hooks"] = _ah

import ml_dtypes  # noqa: E402
import concourse.bass as bass  # noqa: E402
import concourse.bacc as bacc  # noqa: E402
import concourse.tile as tile  # noqa: E402
from contextlib import ExitStack  # noqa: E402
from concourse import mybir  # noqa: E402
from concourse import bass_utils  # noqa: E402
from concourse.masks import make_identity  # noqa: E402

H = 0.5
EPS = 1e-8
B, D, R, O = 4096, 256, 128, 64
NCORES = 8
BL = B // NCORES          # 512 batch rows per core
NT = BL // 128            # 4 partition tiles per core
RO = R * O                # 8192 G columns per row
F32 = mybir.dt.float32
BF16 = mybir.dt.bfloat16
EXPF = mybir.ActivationFunctionType.Exp
ADD = mybir.AluOpType.add

# const blob column offsets (bf16 elements)
OBM, OK2, OW2, OXT = 0, 256, 384, 520
NC_BASE = OXT + 1024          # 1544
OA = NC_BASE                  # A appended when sigmas are non-uniform

_CACHE = {}
LAST_RESULT = None
TRACE = False
TRACE_DIR = "/root/problem/work/trace_out"
# Level-1 tree via SDMA accum_op=add was tried and wedges the device: the
# SWDGE descriptor generator (GpSimd) deadlocks against DVE's shared SBUF
# port (exclusive lock). Keep the level-1 add on DVE.
L1_DMA = os.environ.get("L1_DMA", "0") == "1"


def _build(use_a: bool):
    ncols = NC_BASE + (256 if use_a else 0)
    nc = bacc.Bacc("TRN2", target_bir_lowering=False, debug=False)
    CONST = nc.dram_tensor("CONST", [128, ncols], BF16, kind="ExternalInput")
    WT = nc.dram_tensor("WT", [D, RO], BF16, kind="ExternalInput")
    out = nc.dram_tensor("out", [BL, O], F32, kind="ExternalOutput")

    with tile.TileContext(nc) as tc, ExitStack() as ctx:
        consts = ctx.enter_context(tc.tile_pool(name="consts", bufs=1))
        work = ctx.enter_context(tc.tile_pool(name="work", bufs=2))
        gsbp = ctx.enter_context(tc.tile_pool(name="gsb", bufs=2))
        gap = ctx.enter_context(tc.tile_pool(name="ga", bufs=2))
        gbp = ctx.enter_context(tc.tile_pool(name="gb", bufs=2))
        treep = ctx.enter_context(tc.tile_pool(name="tree", bufs=2))

        # ---- packed const DMA split across both HWDGE queues; Wt streams
        # on sync+gpsimd behind it ----
        conc = consts.tile([128, ncols], BF16, tag="conc")
        half = (ncols // 2) & ~3
        nc.sync.dma_start(out=conc[:, 0:half], in_=CONST[:, 0:half])
        nc.scalar.dma_start(out=conc[:, half:], in_=CONST[:, half:])
        bm_sb = conc[:, OBM:OBM + 256].rearrange("p (c r) -> p c r", r=R)
        k2_sb = conc[0:1, OK2:OK2 + R]
        w2b_sb = conc[:, OW2:OW2 + O + 1]          # [R, 65]: W2+b ++ ones col
        xTv = conc[:, OXT:OXT + 1024].rearrange("p (c j) -> p c j", j=BL)
        if use_a:
            a_sb = conc[:, OA:OA + 256].rearrange("p (c r) -> p c r", r=R)
        identB = consts.tile([128, 128], BF16, tag="idb")
        make_identity(nc, identB)
        ones_sb = consts.tile([1, 128], BF16, tag="ones")
        nc.vector.memset(ones_sb, 1.0)
        # Wt pair-chunk tiles: c0 on sync ring, c1 on gpsimd (SWDGE) ring
        wt_sb = [[None] * 4, [None] * 4]
        for q in range(4):
            for c in range(2):
                t_ = consts.tile([128, 2048], BF16, tag=f"wt{c}{q}")
                eng = nc.sync if c == 0 else nc.gpsimd
                eng.dma_start(out=t_[:, :],
                              in_=WT[c * 128:(c + 1) * 128,
                                     q * 2048:(q + 1) * 2048])
                wt_sb[c][q] = t_
        if use_a:
            x2T = consts.tile([128, 2, BL], BF16, tag="x2T")
            for c in range(2):
                nc.scalar.square(x2T[:, c, :], xTv[:, c, :])

        # per-tile softmax state, alive through phase 2
        e_bf = [consts.tile([128, R], BF16, tag=f"e{t}", name=f"e_{t}")
                for t in range(NT)]
        rs_all = consts.tile([128, NT], F32, tag="rs")
        o2_sb = consts.tile([128, NT * (O + 1)], F32, tag="o2sb")

        # ---- phase 1: logits / exp / e^T / out2 for all tiles ----
        with tc.tile_pool(name="ps_pre", bufs=1, space="PSUM") as ps_pre, \
             tc.tile_pool(name="ps_eT", bufs=2, space="PSUM") as ps_eT:
            m_ps = [ps_pre.tile([128, R], F32, tag=f"m{t}", name=f"m_{t}")
                    for t in range(NT)]                           # 1 bank each
            for t in range(NT):
                bs = slice(t * 128, (t + 1) * 128)
                mt = m_ps[t]
                first = True
                if use_a:
                    for c in range(2):
                        nc.tensor.matmul(mt, lhsT=x2T[:, c, bs],
                                         rhs=a_sb[:, c, :],
                                         start=first, stop=False)
                        first = False
                for c in range(2):
                    nc.tensor.matmul(mt, lhsT=xTv[:, c, bs], rhs=bm_sb[:, c, :],
                                     start=first, stop=False)
                    first = False
                nc.tensor.matmul(mt, lhsT=ones_sb, rhs=k2_sb,
                                 start=False, stop=True)
            for t in range(NT):
                nc.scalar.activation(e_bf[t], m_ps[t], EXPF, bias=0.0, scale=1.0)
                eT_ps = ps_eT.tile([128, 128], BF16, tag="eT")
                nc.tensor.transpose(eT_ps, e_bf[t], identB)
                eT_sb = work.tile([128, 128], BF16, tag="eTsb")
                nc.vector.tensor_copy(eT_sb, eT_ps)
                # out2 reuses tile t's freed m bank (m was consumed by exp)
                nc.tensor.matmul(m_ps[t][:, 0:O + 1], lhsT=eT_sb,
                                 rhs=w2b_sb, start=True, stop=True)
                nc.vector.tensor_copy(o2_sb[:, t * 65:(t + 1) * 65],
                                      m_ps[t][:, 0:O + 1])
                nc.vector.reciprocal(rs_all[:, t:t + 1],
                                     o2_sb[:, t * 65 + O:t * 65 + O + 1])
                # pre-normalize: e <- e/sum_e and out2 <- out2/sum_e, so the
                # per-tile epilogue is a plain add (no tensor_scalar_mul)
                nc.scalar.mul(e_bf[t], e_bf[t], rs_all[:, t:t + 1])
                nc.vector.tensor_scalar_mul(
                    o2_sb[:, t * 65:t * 65 + O], o2_sb[:, t * 65:t * 65 + O],
                    rs_all[:, t:t + 1])
                # HAM warm-up: the ~2us of logits matmuls alone never reach
                # the 3.4us sustained-busy window, so the first G chunk would
                # run at the cold 1.2GHz clock (634ns vs 215ns per matmul).
                # Fill the exp-gated PE gaps with matmuls into this tile's
                # already-consumed m bank (no extra PSUM pressure).
                for w in range(3):
                    nc.tensor.matmul(m_ps[t], lhsT=xTv[:, w % 2, 0:128],
                                     rhs=bm_sb[:, w % 2, :],
                                     start=True, stop=True)


        # ---- phase 2: G matmuls, drains, multiplies, tree ----
        # Emission is software-pipelined: tile t's tree/epilogue instructions
        # are emitted AFTER tile t+1's drains/multiplies so the DVE FIFO never
        # blocks the next tile's multiplies behind a finished tile's tree.
        def reduce_half(z3, t, obase, n, sub):
            # z3: [128, n, 64] view whose o-axis starts at absolute o=obase
            t2 = treep.tile([128, n, 32], BF16, tag=f"t2{sub}",
                            name=f"t2_{t}{sub}")
            nc.vector.tensor_add(t2, z3[:, :, 0:32], z3[:, :, 32:64])
            t3 = treep.tile([128, n, 16], BF16, tag=f"t3{sub}",
                            name=f"t3_{t}{sub}")
            nc.vector.tensor_add(t3, t2[:, :, 0:16], t2[:, :, 16:32])
            t4 = treep.tile([128, n, 8], BF16, tag=f"t4{sub}",
                            name=f"t4_{t}{sub}")
            nc.vector.tensor_add(t4, t3[:, :, 0:8], t3[:, :, 8:16])
            t5 = treep.tile([128, n, 4], BF16, tag=f"t5{sub}",
                            name=f"t5_{t}{sub}")
            nc.vector.tensor_add(t5, t4[:, :, 0:4], t4[:, :, 4:8])
            t6 = treep.tile([128, n, 2], BF16, tag=f"t6{sub}",
                            name=f"t6_{t}{sub}")
            nc.vector.tensor_add(t6, t5[:, :, 0:2], t5[:, :, 2:4])
            red = work.tile([128, n, 1], BF16, tag=f"red{sub}",
                            name=f"red_{t}{sub}")
            nc.vector.tensor_add(red, t6[:, :, 0:1], t6[:, :, 1:2])
            osb = work.tile([128, n], F32, tag=f"osb{sub}", name=f"osb_{t}{sub}")
            nc.vector.tensor_add(osb, red.rearrange("p o () -> p o"),
                                 o2_sb[:, t * 65 + obase:t * 65 + obase + n])
            nc.sync.dma_start(out=out[t * 128:(t + 1) * 128, obase:obase + n],
                              in_=osb)

        with tc.tile_pool(name="ps_g", bufs=2, space="PSUM") as ps_g:
            from collections import deque
            state = {}
            pending = deque()   # generators of deferred tree steps

            def pump(k):
                # emit up to k deferred tree ops into the DVE stream
                steps = 0
                while pending and steps < k:
                    try:
                        next(pending[0])
                        steps += 1
                    except StopIteration:
                        pending.popleft()

            def reduce_steps(z3, t, obase, n, sub):
                # generator form of reduce_half: one DVE op per step
                t2 = treep.tile([128, n, 32], BF16, tag=f"t2{sub}",
                                name=f"t2g_{t}{sub}")
                nc.vector.tensor_add(t2, z3[:, :, 0:32], z3[:, :, 32:64])
                yield
                t3 = treep.tile([128, n, 16], BF16, tag=f"t3{sub}",
                                name=f"t3g_{t}{sub}")
                nc.vector.tensor_add(t3, t2[:, :, 0:16], t2[:, :, 16:32])
                yield
                t4 = treep.tile([128, n, 8], BF16, tag=f"t4{sub}",
                                name=f"t4g_{t}{sub}")
                nc.vector.tensor_add(t4, t3[:, :, 0:8], t3[:, :, 8:16])
                yield
                t5 = treep.tile([128, n, 4], BF16, tag=f"t5{sub}",
                                name=f"t5g_{t}{sub}")
                nc.vector.tensor_add(t5, t4[:, :, 0:4], t4[:, :, 4:8])
                yield
                t6 = treep.tile([128, n, 2], BF16, tag=f"t6{sub}",
                                name=f"t6g_{t}{sub}")
                nc.vector.tensor_add(t6, t5[:, :, 0:2], t5[:, :, 2:4])
                yield
                red = work.tile([128, n, 1], BF16, tag=f"red{sub}",
                                name=f"redg_{t}{sub}")
                nc.vector.tensor_add(red, t6[:, :, 0:1], t6[:, :, 1:2])
                yield
                osb = work.tile([128, n], F32, tag=f"osb{sub}",
                                name=f"osbg_{t}{sub}")
                nc.vector.tensor_add(osb, red.rearrange("p o () -> p o"),
                                     o2_sb[:, t * 65 + obase:t * 65 + obase + n])
                nc.sync.dma_start(
                    out=out[t * 128:(t + 1) * 128, obase:obase + n], in_=osb)
                yield

            def tree_steps(t):
                ga3, gb3 = state["t%d" % t]
                t1 = treep.tile([128, 64, 64], BF16, tag="t1", name=f"t1_{t}")
                nc.vector.tensor_add(t1, ga3, gb3)
                yield
                yield from reduce_steps(t1, t, 0, 64, "")

            def emit_G(t):
                bs = slice(t * 128, (t + 1) * 128)
                last = t == NT - 1
                gsb = gsbp.tile([128, RO], BF16, tag="gsb", name=f"gsb_{t}")
                ga = gap.tile([128, 4096], BF16, tag="ga", name=f"ga_{t}")
                gb = gbp.tile([128, 4096], BF16, tag="gb", name=f"gb_{t}")
                ga3 = ga.rearrange("p (o r) -> p o r", r=64)
                gb3 = gb.rearrange("p (o r) -> p o r", r=64)
                gsb3 = gsb.rearrange("p (o r) -> p o r", r=64)
                # last tile: o-low pair-chunks first so its first half-tree
                # can start while the o-high matmuls still run
                order = (0, 2, 1, 3) if last else (0, 1, 2, 3)
                for i, pq in enumerate(order):
                    gt = ps_g.tile([128, 2048], F32, tag="g", name=f"g_{t}_{pq}")
                    for c in range(2):
                        for h in range(4):
                            nc.tensor.matmul(
                                gt[:, h * 512:(h + 1) * 512],
                                lhsT=xTv[:, c, bs],
                                rhs=wt_sb[c][pq][:, h * 512:(h + 1) * 512],
                                start=(c == 0), stop=(c == 1),
                            )
                    half, oq = divmod(pq, 2)
                    dst3 = (ga3 if half == 0 else gb3)[:, oq * 32:(oq + 1) * 32, :]
                    ebc = (e_bf[t][:, half * 64:(half + 1) * 64]
                           .rearrange("p r -> p () r").broadcast_to((128, 32, 64)))
                    if last and i == 3:
                        # final chunk of the run: multiply straight from PSUM
                        # (1x) to skip the ScalarE drain on the critical tail
                        nc.vector.tensor_mul(
                            dst3,
                            gt.rearrange("p (o r) -> p o r", r=64), ebc)
                    elif t == 0 and i == 0:
                        # pipeline fill: halve the first drain so the first
                        # DVE multiply starts ~1us earlier
                        for hh in range(2):
                            cl = slice(hh * 1024, (hh + 1) * 1024)
                            nc.scalar.copy(gsb[:, cl], gt[:, cl])
                            nc.vector.tensor_mul(
                                dst3[:, hh * 16:(hh + 1) * 16, :],
                                gsb3[:, hh * 16:(hh + 1) * 16, :],
                                (e_bf[t][:, 0:64].rearrange("p r -> p () r")
                                 .broadcast_to((128, 16, 64))))
                    else:
                        nc.scalar.copy(gsb[:, pq * 2048:(pq + 1) * 2048], gt)
                        nc.vector.tensor_mul(
                            dst3, gsb3[:, pq * 32:(pq + 1) * 32, :], ebc)
                    # interleave up to 3 deferred tree ops of the previous
                    # tile into the DVE stream after each chunk's multiply
                    pump(3)
                    if last and i == 1:
                        # o 0:32 complete in both halves: first half-tree now
                        t1a = treep.tile([128, 32, 64], BF16, tag="t1a")
                        nc.vector.tensor_add(t1a, ga3[:, 0:32, :],
                                             gb3[:, 0:32, :])
                        reduce_half(t1a, t, 0, 32, "a")
                state["ga"], state["gb"] = ga3, gb3

            for t in range(NT):
                if t > 0:
                    pending.append(tree_steps(t - 1))
                emit_G(t)
                state["t%d" % t] = (state["ga"], state["gb"])
            pump(10 ** 9)   # flush any remaining deferred steps
            # last tile: second half-tree only (first half emitted inline)
            ga3, gb3 = state["t%d" % (NT - 1)]
            t1b = treep.tile([128, 32, 64], BF16, tag="t1b")
            nc.vector.tensor_add(t1b, ga3[:, 32:64, :], gb3[:, 32:64, :])
            reduce_half(t1b, NT - 1, 32, 32, "b")

    nc.finalize()
    return nc


def _get_nc(use_a: bool):
    key = ("nc", use_a)
    if key not in _CACHE:
        _CACHE[key] = _build(use_a)
    return _CACHE[key]


def _host_prep(centers, sigmas, W, b):
    c64 = centers.astype(np.float64)
    S = (H / sigmas.astype(np.float64) ** 2) + EPS          # (D,R)
    use_a = not np.allclose(S, S.flat[0])
    bf = ml_dtypes.bfloat16
    ncols = NC_BASE + (256 if use_a else 0)
    CB = np.zeros((128, ncols), dtype=bf)
    Bm = (2.0 * S * c64 / D).astype(bf)                      # X coeff
    CB[:, OBM:OBM + 128] = Bm[0:128]
    CB[:, OBM + 128:OBM + 256] = Bm[128:256]
    K2 = (-(S * c64 * c64).sum(axis=0) / D).astype(bf)
    CB[0, OK2:OK2 + R] = K2
    W2b = np.concatenate(
        [W[D * R:].astype(np.float64) + b[None, :].astype(np.float64),
         np.ones((R, 1))], axis=1
    ).astype(bf)
    CB[:, OW2:OW2 + O + 1] = W2b
    if use_a:
        A = (-S / D).astype(bf)
        CB[:, OA:OA + 128] = A[0:128]
        CB[:, OA + 128:OA + 256] = A[128:256]
    W1 = W[: D * R].reshape(2, 64, D, O)          # (half, rr, d, o)
    # split-half o-major: Wt[d, h*4096 + o*64 + rr] = W1[h, rr, d, o]
    Wt = np.ascontiguousarray(
        W1.transpose(2, 0, 3, 1).reshape(D, RO)).astype(bf)
    return use_a, CB, Wt


def kernel(X, centers, sigmas, W, b):
    global LAST_RESULT
    X = np.asarray(X, dtype=np.float32)
    centers = np.asarray(centers, dtype=np.float32)
    sigmas = np.asarray(sigmas, dtype=np.float32)
    W = np.asarray(W, dtype=np.float32)
    b = np.asarray(b, dtype=np.float32)

    use_a, CB, Wt = _host_prep(centers, sigmas, W, b)
    Xb = X.astype(ml_dtypes.bfloat16)
    nc = _get_nc(use_a)
    in_maps = []
    for k in range(NCORES):
        cb = CB.copy()
        xt = Xb[k * BL:(k + 1) * BL].T                      # (D, BL)
        cb[:, OXT:OXT + BL] = xt[0:128]
        cb[:, OXT + BL:OXT + 1024] = xt[128:256]
        in_maps.append({"CONST": cb, "WT": Wt})
    kw = {}
    if TRACE:
        import shutil
        shutil.rmtree(TRACE_DIR, ignore_errors=True)
        kw = {"trace": True, "tmpdir": TRACE_DIR}
    res = bass_utils.run_bass_kernel_spmd(
        nc, in_maps, core_ids=list(range(NCORES)), **kw
    )
    LAST_RESULT = res
    return np.concatenate([res.results[k]["out"] for k in range(NCORES)], axis=0)


# revision 32
# speedup vs baseline: 1.0258x; 1.0043x over previous
"""HTSK fuzzy-system kernel for Trainium2 (Bass/Tile), 8-core data-parallel.

Math (per batch row b):
  S     = H/sigma^2 + EPS                          (D,R)
  m     = mean_d(-(X_bd - C_dr)^2 * S_dr)          (B,R)
        = X^2 @ (-S/D) + X @ (2*S*C/D) + K2        (matmul expansion)
  e     = exp(m)            (unnormalized softmax; m is bounded, no max needed)
  out   = (1/sum_r e) * ( sum_r e_br * G_bro  +  e @ (W2 + 1 b^T) )
  G     = X @ Wt,  Wt[d, h*4096 + o*64 + rr] = W[(h*64+rr)*D+d, o]

G layout: r split in low/high halves (h), o-major within each, rr innermost.
- innermost rr is step-1 so the e-broadcast multiply runs in DVE 2x_1P mode
- the r-halves live in two separate contiguous 4KB-per-partition tiles, so
  tree level 1 is ONE contiguous SBUF->SBUF DMA with the CCE inline adder
  (accum_op=add), running on the otherwise-idle DMA engines
When sigmas are uniform the X^2 term is constant over r and cancels in the
softmax, so the A-matmuls are dropped. sum_r e comes from a ones column
appended to W2.

Two phases per core:
  1) logits/exp/e-transpose/out2 for all 4 row-tiles, using 3 PSUM banks
     (scoped pools), overlapped with the Wt DMA stream
  2) G phase: all 8 PSUM banks as 2x[128,2048] fp32 ping-pong; per
     pair-chunk 8 matmuls (stationary changes once), ScalarE drain to bf16,
     DVE 2x multiply; tree L1 on DMA (CCE), L2..L7 + epilogue on DVE

Sharding: batch B=4096 split 512 rows per core; weights replicated.
All small constants + X^T ship in one packed [128, x] DMA blob per core.
"""
import os
import sys
import types

import numpy as np

sys.path.insert(0, "/opt/trn_rl_repo")

# NTFF profile-hook registry: trn_boot sets it at jax init, concourse
# bass_utils reads it when trace=True. The container's antenv package lacks
# this submodule, so provide it before anything imports jax/concourse.
if "antenv.axon_hooks" not in sys.modules:
    _ah = types.ModuleType("antenv.axon_hooks")
    _ah._hook = None

    def _set_hook(hook):
        _ah._hook = hook

    def _get_hook():
        return _ah._hook

    _ah.set_axon_ntff_profile_hook = _set_hook
    _ah.get_axon_ntff_profile_hook = _get_hook
    sys.modules["antenv.axon_hooks"] = _ah

import ml_dtypes  # noqa: E402
import concourse.bass as bass  # noqa: E402
import concourse.bacc as bacc  # noqa: E402
import concourse.tile as tile  # noqa: E402
from contextlib import ExitStack  # noqa: E402
from concourse import mybir  # noqa: E402
from concourse import bass_utils  # noqa: E402
from concourse.masks import make_identity  # noqa: E402

H = 0.5
EPS = 1e-8
B, D, R, O = 4096, 256, 128, 64
NCORES = 8
BL = B // NCORES          # 512 batch rows per core
NT = BL // 128            # 4 partition tiles per core
RO = R * O                # 8192 G columns per row
F32 = mybir.dt.float32
BF16 = mybir.dt.bfloat16
EXPF = mybir.ActivationFunctionType.Exp
ADD = mybir.AluOpType.add

# const blob column offsets (bf16 elements)
OBM, OK2, OW2, OXT = 0, 256, 384, 520
NC_BASE = OXT + 1024          # 1544
OA = NC_BASE                  # A appended when sigmas are non-uniform

_CACHE = {}
LAST_RESULT = None
TRACE = False
TRACE_DIR = "/root/problem/work/trace_out"
# Level-1 tree via SDMA accum_op=add was tried and wedges the device: the
# SWDGE descriptor generator (GpSimd) deadlocks against DVE's shared SBUF
# port (exclusive lock). Keep the level-1 add on DVE.
L1_DMA = os.environ.get("L1_DMA", "0") == "1"


def _build(use_a: bool):
    ncols = NC_BASE + (256 if use_a else 0)
    nc = bacc.Bacc("TRN2", target_bir_lowering=False, debug=False)
    CONST = nc.dram_tensor("CONST", [128, ncols], BF16, kind="ExternalInput")
    WT = nc.dram_tensor("WT", [D, RO], BF16, kind="ExternalInput")
    out = nc.dram_tensor("out", [BL, O], F32, kind="ExternalOutput")

    with tile.TileContext(nc) as tc, ExitStack() as ctx:
        consts = ctx.enter_context(tc.tile_pool(name="consts", bufs=1))
        work = ctx.enter_context(tc.tile_pool(name="work", bufs=2))
        gsbp = ctx.enter_context(tc.tile_pool(name="gsb", bufs=2))
        gap = ctx.enter_context(tc.tile_pool(name="ga", bufs=2))
        gbp = ctx.enter_context(tc.tile_pool(name="gb", bufs=2))
        treep = ctx.enter_context(tc.tile_pool(name="tree", bufs=2))

        # ---- packed const DMA split across both HWDGE queues; Wt streams
        # on sync+gpsimd behind it ----
        conc = consts.tile([128, ncols], BF16, tag="conc")
        half = (ncols // 2) & ~3
        nc.sync.dma_start(out=conc[:, 0:half], in_=CONST[:, 0:half])
        nc.scalar.dma_start(out=conc[:, half:], in_=CONST[:, half:])
        bm_sb = conc[:, OBM:OBM + 256].rearrange("p (c r) -> p c r", r=R)
        k2_sb = conc[0:1, OK2:OK2 + R]
        w2b_sb = conc[:, OW2:OW2 + O + 1]          # [R, 65]: W2+b ++ ones col
        xTv = conc[:, OXT:OXT + 1024].rearrange("p (c j) -> p c j", j=BL)
        if use_a:
            a_sb = conc[:, OA:OA + 256].rearrange("p (c r) -> p c r", r=R)
        identB = consts.tile([128, 128], BF16, tag="idb")
        make_identity(nc, identB)
        ones_sb = consts.tile([1, 128], BF16, tag="ones")
        nc.vector.memset(ones_sb, 1.0)
        # Wt pair-chunk tiles: c0 on sync ring, c1 on gpsimd (SWDGE) ring
        wt_sb = [[None] * 4, [None] * 4]
        for q in range(4):
            for c in range(2):
                t_ = consts.tile([128, 2048], BF16, tag=f"wt{c}{q}")
                eng = nc.sync if c == 0 else nc.gpsimd
                eng.dma_start(out=t_[:, :],
                              in_=WT[c * 128:(c + 1) * 128,
                                     q * 2048:(q + 1) * 2048])
                wt_sb[c][q] = t_
        if use_a:
            x2T = consts.tile([128, 2, BL], BF16, tag="x2T")
            for c in range(2):
                nc.scalar.square(x2T[:, c, :], xTv[:, c, :])

        # per-tile softmax state, alive through phase 2
        e_bf = [consts.tile([128, R], BF16, tag=f"e{t}", name=f"e_{t}")
                for t in range(NT)]
        rs_all = consts.tile([128, NT], F32, tag="rs")
        o2_sb = consts.tile([128, NT * (O + 1)], F32, tag="o2sb")

        # ---- phase 1: logits / exp / e^T / out2 for all tiles ----
        with tc.tile_pool(name="ps_pre", bufs=1, space="PSUM") as ps_pre, \
             tc.tile_pool(name="ps_eT", bufs=2, space="PSUM") as ps_eT:
            m_ps = [ps_pre.tile([128, R], F32, tag=f"m{t}", name=f"m_{t}")
                    for t in range(NT)]                           # 1 bank each
            for t in range(NT):
                bs = slice(t * 128, (t + 1) * 128)
                mt = m_ps[t]
                first = True
                if use_a:
                    for c in range(2):
                        nc.tensor.matmul(mt, lhsT=x2T[:, c, bs],
                                         rhs=a_sb[:, c, :],
                                         start=first, stop=False)
                        first = False
                for c in range(2):
                    nc.tensor.matmul(mt, lhsT=xTv[:, c, bs], rhs=bm_sb[:, c, :],
                                     start=first, stop=False)
                    first = False
                nc.tensor.matmul(mt, lhsT=ones_sb, rhs=k2_sb,
                                 start=False, stop=True)
            for t in range(NT):
                nc.scalar.activation(e_bf[t], m_ps[t], EXPF, bias=0.0, scale=1.0)
                eT_ps = ps_eT.tile([128, 128], BF16, tag="eT")
                nc.tensor.transpose(eT_ps, e_bf[t], identB)
                eT_sb = work.tile([128, 128], BF16, tag="eTsb")
                nc.vector.tensor_copy(eT_sb, eT_ps)
                # out2 reuses tile t's freed m bank (m was consumed by exp)
                nc.tensor.matmul(m_ps[t][:, 0:O + 1], lhsT=eT_sb,
                                 rhs=w2b_sb, start=True, stop=True)
                nc.vector.tensor_copy(o2_sb[:, t * 65:(t + 1) * 65],
                                      m_ps[t][:, 0:O + 1])
                nc.vector.reciprocal(rs_all[:, t:t + 1],
                                     o2_sb[:, t * 65 + O:t * 65 + O + 1])
                # pre-normalize: e <- e/sum_e and out2 <- out2/sum_e, so the
                # per-tile epilogue is a plain add (no tensor_scalar_mul)
                nc.scalar.mul(e_bf[t], e_bf[t], rs_all[:, t:t + 1])
                nc.vector.tensor_scalar_mul(
                    o2_sb[:, t * 65:t * 65 + O], o2_sb[:, t * 65:t * 65 + O],
                    rs_all[:, t:t + 1])



        # ---- phase 2: G matmuls, drains, multiplies, tree ----
        # Emission is software-pipelined: tile t's tree/epilogue instructions
        # are emitted AFTER tile t+1's drains/multiplies so the DVE FIFO never
        # blocks the next tile's multiplies behind a finished tile's tree.
        def reduce_half(z3, t, obase, n, sub):
            # z3: [128, n, 64] view whose o-axis starts at absolute o=obase
            t2 = treep.tile([128, n, 32], BF16, tag=f"t2{sub}",
                            name=f"t2_{t}{sub}")
            nc.vector.tensor_add(t2, z3[:, :, 0:32], z3[:, :, 32:64])
            t3 = treep.tile([128, n, 16], BF16, tag=f"t3{sub}",
                            name=f"t3_{t}{sub}")
            nc.vector.tensor_add(t3, t2[:, :, 0:16], t2[:, :, 16:32])
            t4 = treep.tile([128, n, 8], BF16, tag=f"t4{sub}",
                            name=f"t4_{t}{sub}")
            nc.vector.tensor_add(t4, t3[:, :, 0:8], t3[:, :, 8:16])
            t5 = treep.tile([128, n, 4], BF16, tag=f"t5{sub}",
                            name=f"t5_{t}{sub}")
            nc.vector.tensor_add(t5, t4[:, :, 0:4], t4[:, :, 4:8])
            t6 = treep.tile([128, n, 2], BF16, tag=f"t6{sub}",
                            name=f"t6_{t}{sub}")
            nc.vector.tensor_add(t6, t5[:, :, 0:2], t5[:, :, 2:4])
            red = work.tile([128, n, 1], BF16, tag=f"red{sub}",
                            name=f"red_{t}{sub}")
            nc.vector.tensor_add(red, t6[:, :, 0:1], t6[:, :, 1:2])
            osb = work.tile([128, n], F32, tag=f"osb{sub}", name=f"osb_{t}{sub}")
            nc.vector.tensor_add(osb, red.rearrange("p o () -> p o"),
                                 o2_sb[:, t * 65 + obase:t * 65 + obase + n])
            nc.sync.dma_start(out=out[t * 128:(t + 1) * 128, obase:obase + n],
                              in_=osb)

        with tc.tile_pool(name="ps_g", bufs=2, space="PSUM") as ps_g:
            from collections import deque
            state = {}
            pending = deque()   # generators of deferred tree steps

            def pump(k):
                # emit up to k deferred tree ops into the DVE stream
                steps = 0
                while pending and steps < k:
                    try:
                        next(pending[0])
                        steps += 1
                    except StopIteration:
                        pending.popleft()

            def reduce_steps(z3, t, obase, n, sub):
                # generator form of reduce_half: one DVE op per step
                t2 = treep.tile([128, n, 32], BF16, tag=f"t2{sub}",
                                name=f"t2g_{t}{sub}")
                nc.vector.tensor_add(t2, z3[:, :, 0:32], z3[:, :, 32:64])
                yield
                t3 = treep.tile([128, n, 16], BF16, tag=f"t3{sub}",
                                name=f"t3g_{t}{sub}")
                nc.vector.tensor_add(t3, t2[:, :, 0:16], t2[:, :, 16:32])
                yield
                t4 = treep.tile([128, n, 8], BF16, tag=f"t4{sub}",
                                name=f"t4g_{t}{sub}")
                nc.vector.tensor_add(t4, t3[:, :, 0:8], t3[:, :, 8:16])
                yield
                t5 = treep.tile([128, n, 4], BF16, tag=f"t5{sub}",
                                name=f"t5g_{t}{sub}")
                nc.vector.tensor_add(t5, t4[:, :, 0:4], t4[:, :, 4:8])
                yield
                t6 = treep.tile([128, n, 2], BF16, tag=f"t6{sub}",
                                name=f"t6g_{t}{sub}")
                nc.vector.tensor_add(t6, t5[:, :, 0:2], t5[:, :, 2:4])
                yield
                red = work.tile([128, n, 1], BF16, tag=f"red{sub}",
                                name=f"redg_{t}{sub}")
                nc.vector.tensor_add(red, t6[:, :, 0:1], t6[:, :, 1:2])
                yield
                osb = work.tile([128, n], F32, tag=f"osb{sub}",
                                name=f"osbg_{t}{sub}")
                nc.vector.tensor_add(osb, red.rearrange("p o () -> p o"),
                                     o2_sb[:, t * 65 + obase:t * 65 + obase + n])
                nc.sync.dma_start(
                    out=out[t * 128:(t + 1) * 128, obase:obase + n], in_=osb)
                yield

            def tree_steps(t):
                ga3, gb3 = state["t%d" % t]
                t1 = treep.tile([128, 64, 64], BF16, tag="t1", name=f"t1_{t}")
                nc.vector.tensor_add(t1, ga3, gb3)
                yield
                yield from reduce_steps(t1, t, 0, 64, "")

            def emit_G(t):
                bs = slice(t * 128, (t + 1) * 128)
                last = t == NT - 1
                gsb = gsbp.tile([128, RO], BF16, tag="gsb", name=f"gsb_{t}")
                ga = gap.tile([128, 4096], BF16, tag="ga", name=f"ga_{t}")
                gb = gbp.tile([128, 4096], BF16, tag="gb", name=f"gb_{t}")
                ga3 = ga.rearrange("p (o r) -> p o r", r=64)
                gb3 = gb.rearrange("p (o r) -> p o r", r=64)
                gsb3 = gsb.rearrange("p (o r) -> p o r", r=64)
                # last tile: o-low pair-chunks first so its first half-tree
                # can start while the o-high matmuls still run
                order = (0, 2, 1, 3) if last else (0, 1, 2, 3)
                for i, pq in enumerate(order):
                    gt = ps_g.tile([128, 2048], F32, tag="g", name=f"g_{t}_{pq}")
                    for c in range(2):
                        for h in range(4):
                            nc.tensor.matmul(
                                gt[:, h * 512:(h + 1) * 512],
                                lhsT=xTv[:, c, bs],
                                rhs=wt_sb[c][pq][:, h * 512:(h + 1) * 512],
                                start=(c == 0), stop=(c == 1),
                            )
                    half, oq = divmod(pq, 2)
                    dst3 = (ga3 if half == 0 else gb3)[:, oq * 32:(oq + 1) * 32, :]
                    ebc = (e_bf[t][:, half * 64:(half + 1) * 64]
                           .rearrange("p r -> p () r").broadcast_to((128, 32, 64)))
                    if last and i == 3:
                        # final chunk of the run: multiply straight from PSUM
                        # (1x) to skip the ScalarE drain on the critical tail
                        nc.vector.tensor_mul(
                            dst3,
                            gt.rearrange("p (o r) -> p o r", r=64), ebc)
                    elif t == 0 and i == 0:
                        # pipeline fill: halve the first drain so the first
                        # DVE multiply starts ~1us earlier
                        for hh in range(2):
                            cl = slice(hh * 1024, (hh + 1) * 1024)
                            nc.scalar.copy(gsb[:, cl], gt[:, cl])
                            nc.vector.tensor_mul(
                                dst3[:, hh * 16:(hh + 1) * 16, :],
                                gsb3[:, hh * 16:(hh + 1) * 16, :],
                                (e_bf[t][:, 0:64].rearrange("p r -> p () r")
                                 .broadcast_to((128, 16, 64))))
                    else:
                        nc.scalar.copy(gsb[:, pq * 2048:(pq + 1) * 2048], gt)
                        nc.vector.tensor_mul(
                            dst3, gsb3[:, pq * 32:(pq + 1) * 32, :], ebc)
                    # interleave up to 3 deferred tree ops of the previous
                    # tile into the DVE stream after each chunk's multiply
                    pump(3)
                    if last and i == 1:
                        # o 0:32 complete in both halves: first half-tree now
                        t1a = treep.tile([128, 32, 64], BF16, tag="t1a")
                        nc.vector.tensor_add(t1a, ga3[:, 0:32, :],
                                             gb3[:, 0:32, :])
                        reduce_half(t1a, t, 0, 32, "a")
                state["ga"], state["gb"] = ga3, gb3

            for t in range(NT):
                if t > 0:
                    pending.append(tree_steps(t - 1))
                emit_G(t)
                state["t%d" % t] = (state["ga"], state["gb"])
            pump(10 ** 9)   # flush any remaining deferred steps
            # last tile: second half-tree only (first half emitted inline)
            ga3, gb3 = state["t%d" % (NT - 1)]
            t1b = treep.tile([128, 32, 64], BF16, tag="t1b")
            nc.vector.tensor_add(t1b, ga3[:, 32:64, :], gb3[:, 32:64, :])
            reduce_half(t1b, NT - 1, 32, 32, "b")

    nc.finalize()
    return nc


def _get_nc(use_a: bool):
    key = ("nc", use_a)
    if key not in _CACHE:
        _CACHE[key] = _build(use_a)
    return _CACHE[key]


def _host_prep(centers, sigmas, W, b):
    c64 = centers.astype(np.float64)
    S = (H / sigmas.astype(np.float64) ** 2) + EPS          # (D,R)
    use_a = not np.allclose(S, S.flat[0])
    bf = ml_dtypes.bfloat16
    ncols = NC_BASE + (256 if use_a else 0)
    CB = np.zeros((128, ncols), dtype=bf)
    Bm = (2.0 * S * c64 / D).astype(bf)                      # X coeff
    CB[:, OBM:OBM + 128] = Bm[0:128]
    CB[:, OBM + 128:OBM + 256] = Bm[128:256]
    K2 = (-(S * c64 * c64).sum(axis=0) / D).astype(bf)
    CB[0, OK2:OK2 + R] = K2
    W2b = np.concatenate(
        [W[D * R:].astype(np.float64) + b[None, :].astype(np.float64),
         np.ones((R, 1))], axis=1
    ).astype(bf)
    CB[:, OW2:OW2 + O + 1] = W2b
    if use_a:
        A = (-S / D).astype(bf)
        CB[:, OA:OA + 128] = A[0:128]
        CB[:, OA + 128:OA + 256] = A[128:256]
    W1 = W[: D * R].reshape(2, 64, D, O)          # (half, rr, d, o)
    # split-half o-major: Wt[d, h*4096 + o*64 + rr] = W1[h, rr, d, o]
    Wt = np.ascontiguousarray(
        W1.transpose(2, 0, 3, 1).reshape(D, RO)).astype(bf)
    return use_a, CB, Wt


def kernel(X, centers, sigmas, W, b):
    global LAST_RESULT
    X = np.asarray(X, dtype=np.float32)
    centers = np.asarray(centers, dtype=np.float32)
    sigmas = np.asarray(sigmas, dtype=np.float32)
    W = np.asarray(W, dtype=np.float32)
    b = np.asarray(b, dtype=np.float32)

    use_a, CB, Wt = _host_prep(centers, sigmas, W, b)
    Xb = X.astype(ml_dtypes.bfloat16)
    nc = _get_nc(use_a)
    in_maps = []
    for k in range(NCORES):
        cb = CB.copy()
        xt = Xb[k * BL:(k + 1) * BL].T                      # (D, BL)
        cb[:, OXT:OXT + BL] = xt[0:128]
        cb[:, OXT + BL:OXT + 1024] = xt[128:256]
        in_maps.append({"CONST": cb, "WT": Wt})
    kw = {}
    if TRACE:
        import shutil
        shutil.rmtree(TRACE_DIR, ignore_errors=True)
        kw = {"trace": True, "tmpdir": TRACE_DIR}
    res = bass_utils.run_bass_kernel_spmd(
        nc, in_maps, core_ids=list(range(NCORES)), **kw
    )
    LAST_RESULT = res
    return np.concatenate([res.results[k]["out"] for k in range(NCORES)], axis=0)


# revision 33
# speedup vs baseline: 1.0302x; 1.0043x over previous
"""HTSK fuzzy-system kernel for Trainium2 (Bass/Tile), 8-core data-parallel.

Math (per batch row b):
  S     = H/sigma^2 + EPS                          (D,R)
  m     = mean_d(-(X_bd - C_dr)^2 * S_dr)          (B,R)
        = X^2 @ (-S/D) + X @ (2*S*C/D) + K2        (matmul expansion)
  e     = exp(m)            (unnormalized softmax; m is bounded, no max needed)
  out   = (1/sum_r e) * ( sum_r e_br * G_bro  +  e @ (W2 + 1 b^T) )
  G     = X @ Wt,  Wt[d, h*4096 + o*64 + rr] = W[(h*64+rr)*D+d, o]

G layout: r split in low/high halves (h), o-major within each, rr innermost.
- innermost rr is step-1 so the e-broadcast multiply runs in DVE 2x_1P mode
- the r-halves live in two separate contiguous 4KB-per-partition tiles, so
  tree level 1 is ONE contiguous SBUF->SBUF DMA with the CCE inline adder
  (accum_op=add), running on the otherwise-idle DMA engines
When sigmas are uniform the X^2 term is constant over r and cancels in the
softmax, so the A-matmuls are dropped. sum_r e comes from a ones column
appended to W2.

Two phases per core:
  1) logits/exp/e-transpose/out2 for all 4 row-tiles, using 3 PSUM banks
     (scoped pools), overlapped with the Wt DMA stream
  2) G phase: all 8 PSUM banks as 2x[128,2048] fp32 ping-pong; per
     pair-chunk 8 matmuls (stationary changes once), ScalarE drain to bf16,
     DVE 2x multiply; tree L1 on DMA (CCE), L2..L7 + epilogue on DVE

Sharding: batch B=4096 split 512 rows per core; weights replicated.
All small constants + X^T ship in one packed [128, x] DMA blob per core.
"""
import os
import sys
import types

import numpy as np

sys.path.insert(0, "/opt/trn_rl_repo")

# NTFF profile-hook registry: trn_boot sets it at jax init, concourse
# bass_utils reads it when trace=True. The container's antenv package lacks
# this submodule, so provide it before anything imports jax/concourse.
if "antenv.axon_hooks" not in sys.modules:
    _ah = types.ModuleType("antenv.axon_hooks")
    _ah._hook = None

    def _set_hook(hook):
        _ah._hook = hook

    def _get_hook():
        return _ah._hook

    _ah.set_axon_ntff_profile_hook = _set_hook
    _ah.get_axon_ntff_profile_hook = _get_hook
    sys.modules["antenv.axon_hooks"] = _ah

import ml_dtypes  # noqa: E402
import concourse.bass as bass  # noqa: E402
import concourse.bacc as bacc  # noqa: E402
import concourse.tile as tile  # noqa: E402
from contextlib import ExitStack  # noqa: E402
from concourse import mybir  # noqa: E402
from concourse import bass_utils  # noqa: E402
from concourse.masks import make_identity  # noqa: E402

H = 0.5
EPS = 1e-8
B, D, R, O = 4096, 256, 128, 64
NCORES = 8
BL = B // NCORES          # 512 batch rows per core
NT = BL // 128            # 4 partition tiles per core
RO = R * O                # 8192 G columns per row
F32 = mybir.dt.float32
BF16 = mybir.dt.bfloat16
EXPF = mybir.ActivationFunctionType.Exp
ADD = mybir.AluOpType.add

# const blob column offsets (bf16 elements)
OBM, OK2, OW2, OXT = 0, 256, 384, 520
NC_BASE = OXT + 1024          # 1544
OA = NC_BASE                  # A appended when sigmas are non-uniform

_CACHE = {}
LAST_RESULT = None
TRACE = False
TRACE_DIR = "/root/problem/work/trace_out"
# Level-1 tree via SDMA accum_op=add was tried and wedges the device: the
# SWDGE descriptor generator (GpSimd) deadlocks against DVE's shared SBUF
# port (exclusive lock). Keep the level-1 add on DVE.
L1_DMA = os.environ.get("L1_DMA", "0") == "1"


def _build(use_a: bool):
    ncols = NC_BASE + (256 if use_a else 0)
    nc = bacc.Bacc("TRN2", target_bir_lowering=False, debug=False)
    CONST = nc.dram_tensor("CONST", [128, ncols], BF16, kind="ExternalInput")
    WT = nc.dram_tensor("WT", [D, RO], BF16, kind="ExternalInput")
    out = nc.dram_tensor("out", [BL, O], F32, kind="ExternalOutput")

    with tile.TileContext(nc) as tc, ExitStack() as ctx:
        consts = ctx.enter_context(tc.tile_pool(name="consts", bufs=1))
        work = ctx.enter_context(tc.tile_pool(name="work", bufs=2))
        gsbp = ctx.enter_context(tc.tile_pool(name="gsb", bufs=2))
        gap = ctx.enter_context(tc.tile_pool(name="ga", bufs=2))
        gbp = ctx.enter_context(tc.tile_pool(name="gb", bufs=2))
        treep = ctx.enter_context(tc.tile_pool(name="tree", bufs=2))

        # ---- packed const DMA split across both HWDGE queues; Wt streams
        # on sync+gpsimd behind it ----
        conc = consts.tile([128, ncols], BF16, tag="conc")
        half = (ncols // 2) & ~3
        nc.sync.dma_start(out=conc[:, 0:half], in_=CONST[:, 0:half])
        nc.scalar.dma_start(out=conc[:, half:], in_=CONST[:, half:])
        bm_sb = conc[:, OBM:OBM + 256].rearrange("p (c r) -> p c r", r=R)
        k2_sb = conc[0:1, OK2:OK2 + R]
        w2b_sb = conc[:, OW2:OW2 + O + 1]          # [R, 65]: W2+b ++ ones col
        xTv = conc[:, OXT:OXT + 1024].rearrange("p (c j) -> p c j", j=BL)
        if use_a:
            a_sb = conc[:, OA:OA + 256].rearrange("p (c r) -> p c r", r=R)
        identB = consts.tile([128, 128], BF16, tag="idb")
        make_identity(nc, identB)
        ones_sb = consts.tile([1, 128], BF16, tag="ones")
        nc.vector.memset(ones_sb, 1.0)
        # Wt pair-chunk tiles: c0 on sync ring, c1 on gpsimd (SWDGE) ring
        wt_sb = [[None] * 4, [None] * 4]
        for q in range(4):
            for c in range(2):
                t_ = consts.tile([128, 2048], BF16, tag=f"wt{c}{q}")
                eng = nc.sync if c == 0 else nc.gpsimd
                eng.dma_start(out=t_[:, :],
                              in_=WT[c * 128:(c + 1) * 128,
                                     q * 2048:(q + 1) * 2048])
                wt_sb[c][q] = t_
        if use_a:
            x2T = consts.tile([128, 2, BL], BF16, tag="x2T")
            for c in range(2):
                nc.scalar.square(x2T[:, c, :], xTv[:, c, :])

        # per-tile softmax state, alive through phase 2
        e_bf = [consts.tile([128, R], BF16, tag=f"e{t}", name=f"e_{t}")
                for t in range(NT)]
        rs_all = consts.tile([128, NT], F32, tag="rs")
        o2_sb = consts.tile([128, NT * (O + 1)], F32, tag="o2sb")

        # ---- phase 1: logits / exp / e^T / out2 for all tiles ----
        with tc.tile_pool(name="ps_pre", bufs=1, space="PSUM") as ps_pre, \
             tc.tile_pool(name="ps_eT", bufs=2, space="PSUM") as ps_eT:
            m_ps = [ps_pre.tile([128, R], F32, tag=f"m{t}", name=f"m_{t}")
                    for t in range(NT)]                           # 1 bank each
            for t in range(NT):
                bs = slice(t * 128, (t + 1) * 128)
                mt = m_ps[t]
                first = True
                if use_a:
                    for c in range(2):
                        nc.tensor.matmul(mt, lhsT=x2T[:, c, bs],
                                         rhs=a_sb[:, c, :],
                                         start=first, stop=False)
                        first = False
                for c in range(2):
                    nc.tensor.matmul(mt, lhsT=xTv[:, c, bs], rhs=bm_sb[:, c, :],
                                     start=first, stop=False)
                    first = False
                nc.tensor.matmul(mt, lhsT=ones_sb, rhs=k2_sb,
                                 start=False, stop=True)
            for t in range(NT):
                nc.scalar.activation(e_bf[t], m_ps[t], EXPF, bias=0.0, scale=1.0)
                eT_ps = ps_eT.tile([128, 128], BF16, tag="eT")
                nc.tensor.transpose(eT_ps, e_bf[t], identB)
                eT_sb = work.tile([128, 128], BF16, tag="eTsb")
                nc.vector.tensor_copy(eT_sb, eT_ps)
                # out2 reuses tile t's freed m bank (m was consumed by exp)
                nc.tensor.matmul(m_ps[t][:, 0:O + 1], lhsT=eT_sb,
                                 rhs=w2b_sb, start=True, stop=True)
                nc.vector.tensor_copy(o2_sb[:, t * 65:(t + 1) * 65],
                                      m_ps[t][:, 0:O + 1])
                nc.vector.reciprocal(rs_all[:, t:t + 1],
                                     o2_sb[:, t * 65 + O:t * 65 + O + 1])
                # pre-normalize: e <- e/sum_e and out2 <- out2/sum_e, so the
                # per-tile epilogue is a plain add (no tensor_scalar_mul)
                nc.scalar.mul(e_bf[t], e_bf[t], rs_all[:, t:t + 1])
                nc.vector.tensor_scalar_mul(
                    o2_sb[:, t * 65:t * 65 + O], o2_sb[:, t * 65:t * 65 + O],
                    rs_all[:, t:t + 1])



        # ---- phase 2: G matmuls, drains, multiplies, tree ----
        # Emission is software-pipelined: tile t's tree/epilogue instructions
        # are emitted AFTER tile t+1's drains/multiplies so the DVE FIFO never
        # blocks the next tile's multiplies behind a finished tile's tree.
        def reduce_half(z3, t, obase, n, sub):
            # z3: [128, n, 64] view whose o-axis starts at absolute o=obase
            t2 = treep.tile([128, n, 32], BF16, tag=f"t2{sub}",
                            name=f"t2_{t}{sub}")
            nc.vector.tensor_add(t2, z3[:, :, 0:32], z3[:, :, 32:64])
            t3 = treep.tile([128, n, 16], BF16, tag=f"t3{sub}",
                            name=f"t3_{t}{sub}")
            nc.vector.tensor_add(t3, t2[:, :, 0:16], t2[:, :, 16:32])
            t4 = treep.tile([128, n, 8], BF16, tag=f"t4{sub}",
                            name=f"t4_{t}{sub}")
            nc.vector.tensor_add(t4, t3[:, :, 0:8], t3[:, :, 8:16])
            t5 = treep.tile([128, n, 4], BF16, tag=f"t5{sub}",
                            name=f"t5_{t}{sub}")
            nc.vector.tensor_add(t5, t4[:, :, 0:4], t4[:, :, 4:8])
            t6 = treep.tile([128, n, 2], BF16, tag=f"t6{sub}",
                            name=f"t6_{t}{sub}")
            nc.vector.tensor_add(t6, t5[:, :, 0:2], t5[:, :, 2:4])
            red = work.tile([128, n, 1], BF16, tag=f"red{sub}",
                            name=f"red_{t}{sub}")
            nc.vector.tensor_add(red, t6[:, :, 0:1], t6[:, :, 1:2])
            osb = work.tile([128, n], F32, tag=f"osb{sub}", name=f"osb_{t}{sub}")
            nc.vector.tensor_add(osb, red.rearrange("p o () -> p o"),
                                 o2_sb[:, t * 65 + obase:t * 65 + obase + n])
            nc.sync.dma_start(out=out[t * 128:(t + 1) * 128, obase:obase + n],
                              in_=osb)

        with tc.tile_pool(name="ps_g", bufs=2, space="PSUM") as ps_g:
            from collections import deque
            state = {}
            pending = deque()   # generators of deferred tree steps

            def pump(k):
                # emit up to k deferred tree ops into the DVE stream
                steps = 0
                while pending and steps < k:
                    try:
                        next(pending[0])
                        steps += 1
                    except StopIteration:
                        pending.popleft()

            def reduce_steps(z3, t, obase, n, sub):
                # generator form of reduce_half: one DVE op per step
                t2 = treep.tile([128, n, 32], BF16, tag=f"t2{sub}",
                                name=f"t2g_{t}{sub}")
                nc.vector.tensor_add(t2, z3[:, :, 0:32], z3[:, :, 32:64])
                yield
                t3 = treep.tile([128, n, 16], BF16, tag=f"t3{sub}",
                                name=f"t3g_{t}{sub}")
                nc.vector.tensor_add(t3, t2[:, :, 0:16], t2[:, :, 16:32])
                yield
                t4 = treep.tile([128, n, 8], BF16, tag=f"t4{sub}",
                                name=f"t4g_{t}{sub}")
                nc.vector.tensor_add(t4, t3[:, :, 0:8], t3[:, :, 8:16])
                yield
                t5 = treep.tile([128, n, 4], BF16, tag=f"t5{sub}",
                                name=f"t5g_{t}{sub}")
                nc.vector.tensor_add(t5, t4[:, :, 0:4], t4[:, :, 4:8])
                yield
                t6 = treep.tile([128, n, 2], BF16, tag=f"t6{sub}",
                                name=f"t6g_{t}{sub}")
                nc.vector.tensor_add(t6, t5[:, :, 0:2], t5[:, :, 2:4])
                yield
                red = work.tile([128, n, 1], BF16, tag=f"red{sub}",
                                name=f"redg_{t}{sub}")
                nc.vector.tensor_add(red, t6[:, :, 0:1], t6[:, :, 1:2])
                yield
                osb = work.tile([128, n], F32, tag=f"osb{sub}",
                                name=f"osbg_{t}{sub}")
                nc.vector.tensor_add(osb, red.rearrange("p o () -> p o"),
                                     o2_sb[:, t * 65 + obase:t * 65 + obase + n])
                nc.sync.dma_start(
                    out=out[t * 128:(t + 1) * 128, obase:obase + n], in_=osb)
                yield

            def tree_steps(t):
                ga3, gb3 = state["t%d" % t]
                t1 = treep.tile([128, 64, 64], BF16, tag="t1", name=f"t1_{t}")
                nc.vector.tensor_add(t1, ga3, gb3)
                yield
                yield from reduce_steps(t1, t, 0, 64, "")

            def emit_G(t):
                bs = slice(t * 128, (t + 1) * 128)
                last = t == NT - 1
                gsb = gsbp.tile([128, RO], BF16, tag="gsb", name=f"gsb_{t}")
                ga = gap.tile([128, 4096], BF16, tag="ga", name=f"ga_{t}")
                gb = gbp.tile([128, 4096], BF16, tag="gb", name=f"gb_{t}")
                ga3 = ga.rearrange("p (o r) -> p o r", r=64)
                gb3 = gb.rearrange("p (o r) -> p o r", r=64)
                gsb3 = gsb.rearrange("p (o r) -> p o r", r=64)
                # last tile: o-low pair-chunks first so its first half-tree
                # can start while the o-high matmuls still run
                order = (0, 2, 1, 3) if last else (0, 1, 2, 3)
                for i, pq in enumerate(order):
                    gt = ps_g.tile([128, 2048], F32, tag="g", name=f"g_{t}_{pq}")
                    for c in range(2):
                        for h in range(4):
                            nc.tensor.matmul(
                                gt[:, h * 512:(h + 1) * 512],
                                lhsT=xTv[:, c, bs],
                                rhs=wt_sb[c][pq][:, h * 512:(h + 1) * 512],
                                start=(c == 0), stop=(c == 1),
                            )
                    half, oq = divmod(pq, 2)
                    dst3 = (ga3 if half == 0 else gb3)[:, oq * 32:(oq + 1) * 32, :]
                    ebc = (e_bf[t][:, half * 64:(half + 1) * 64]
                           .rearrange("p r -> p () r").broadcast_to((128, 32, 64)))
                    # note: a "multiply straight from PSUM" shortcut for the
                    # last chunk was tried and reverted — DVE runs 100% packed,
                    # so its 1x-mode cost (+1.07us of DVE work) directly adds
                    # to the end time, while the ScalarE drain it saved was
                    # free (ScalarE is idle by then)
                    if t == 0 and i == 0:
                        # pipeline fill: halve the first drain so the first
                        # DVE multiply starts ~1us earlier
                        for hh in range(2):
                            cl = slice(hh * 1024, (hh + 1) * 1024)
                            nc.scalar.copy(gsb[:, cl], gt[:, cl])
                            nc.vector.tensor_mul(
                                dst3[:, hh * 16:(hh + 1) * 16, :],
                                gsb3[:, hh * 16:(hh + 1) * 16, :],
                                (e_bf[t][:, 0:64].rearrange("p r -> p () r")
                                 .broadcast_to((128, 16, 64))))
                    else:
                        nc.scalar.copy(gsb[:, pq * 2048:(pq + 1) * 2048], gt)
                        nc.vector.tensor_mul(
                            dst3, gsb3[:, pq * 32:(pq + 1) * 32, :], ebc)
                    # interleave up to 3 deferred tree ops of the previous
                    # tile into the DVE stream after each chunk's multiply
                    pump(3)
                    if last and i == 1:
                        # o 0:32 complete in both halves: first half-tree now
                        t1a = treep.tile([128, 32, 64], BF16, tag="t1a")
                        nc.vector.tensor_add(t1a, ga3[:, 0:32, :],
                                             gb3[:, 0:32, :])
                        reduce_half(t1a, t, 0, 32, "a")
                state["ga"], state["gb"] = ga3, gb3

            for t in range(NT):
                if t > 0:
                    pending.append(tree_steps(t - 1))
                emit_G(t)
                state["t%d" % t] = (state["ga"], state["gb"])
            pump(10 ** 9)   # flush any remaining deferred steps
            # last tile: second half-tree only (first half emitted inline)
            ga3, gb3 = state["t%d" % (NT - 1)]
            t1b = treep.tile([128, 32, 64], BF16, tag="t1b")
            nc.vector.tensor_add(t1b, ga3[:, 32:64, :], gb3[:, 32:64, :])
            reduce_half(t1b, NT - 1, 32, 32, "b")

    nc.finalize()
    return nc


def _get_nc(use_a: bool):
    key = ("nc", use_a)
    if key not in _CACHE:
        _CACHE[key] = _build(use_a)
    return _CACHE[key]


def _host_prep(centers, sigmas, W, b):
    c64 = centers.astype(np.float64)
    S = (H / sigmas.astype(np.float64) ** 2) + EPS          # (D,R)
    use_a = not np.allclose(S, S.flat[0])
    bf = ml_dtypes.bfloat16
    ncols = NC_BASE + (256 if use_a else 0)
    CB = np.zeros((128, ncols), dtype=bf)
    Bm = (2.0 * S * c64 / D).astype(bf)                      # X coeff
    CB[:, OBM:OBM + 128] = Bm[0:128]
    CB[:, OBM + 128:OBM + 256] = Bm[128:256]
    K2 = (-(S * c64 * c64).sum(axis=0) / D).astype(bf)
    CB[0, OK2:OK2 + R] = K2
    W2b = np.concatenate(
        [W[D * R:].astype(np.float64) + b[None, :].astype(np.float64),
         np.ones((R, 1))], axis=1
    ).astype(bf)
    CB[:, OW2:OW2 + O + 1] = W2b
    if use_a:
        A = (-S / D).astype(bf)
        CB[:, OA:OA + 128] = A[0:128]
        CB[:, OA + 128:OA + 256] = A[128:256]
    W1 = W[: D * R].reshape(2, 64, D, O)          # (half, rr, d, o)
    # split-half o-major: Wt[d, h*4096 + o*64 + rr] = W1[h, rr, d, o]
    Wt = np.ascontiguousarray(
        W1.transpose(2, 0, 3, 1).reshape(D, RO)).astype(bf)
    return use_a, CB, Wt


def kernel(X, centers, sigmas, W, b):
    global LAST_RESULT
    X = np.asarray(X, dtype=np.float32)
    centers = np.asarray(centers, dtype=np.float32)
    sigmas = np.asarray(sigmas, dtype=np.float32)
    W = np.asarray(W, dtype=np.float32)
    b = np.asarray(b, dtype=np.float32)

    use_a, CB, Wt = _host_prep(centers, sigmas, W, b)
    Xb = X.astype(ml_dtypes.bfloat16)
    nc = _get_nc(use_a)
    in_maps = []
    for k in range(NCORES):
        cb = CB.copy()
        xt = Xb[k * BL:(k + 1) * BL].T                      # (D, BL)
        cb[:, OXT:OXT + BL] = xt[0:128]
        cb[:, OXT + BL:OXT + 1024] = xt[128:256]
        in_maps.append({"CONST": cb, "WT": Wt})
    kw = {}
    if TRACE:
        import shutil
        shutil.rmtree(TRACE_DIR, ignore_errors=True)
        kw = {"trace": True, "tmpdir": TRACE_DIR}
    res = bass_utils.run_bass_kernel_spmd(
        nc, in_maps, core_ids=list(range(NCORES)), **kw
    )
    LAST_RESULT = res
    return np.concatenate([res.results[k]["out"] for k in range(NCORES)], axis=0)
